# revision 26
# baseline (speedup 1.0000x reference)
"""MemoryBank kernel v4 for 8x TRN2 NeuronCores (SPMD, batch-parallel).

Same folded algebra as v3 (split-precision scores, exp-based gate fold):

    x  = xh (fp16, DMA'd) + xl (e4m3 * 2^-10, DMA'd)
    S  = xh@Gh  (fp16 full-rate)
       + 2^-10 * [ (xl*2^10)@e4m3(Gh) + e4m3(xh)@(Gl*2^10) ]   (fp8 DoubleRow)
    en = exp(-gate_logit); g = 1/(1+en); Ec' = -E*r*en
    PSUM = xh + Ec'@WvN   (identity-pass + retrieval on PE)
    out  = g * PSUM       (single DVE op per element, fp16 out)

v4 changes vs v3 (227 us):
  - xh8 derived ON-CHIP (8 chunks gpsimd + 8 chunks ACT converting copies)
    instead of DMA'd: HBM traffic 48.8 -> 40.3 MiB.
  - pre-tiled DRAM layouts: every stream is [128 part, contiguous] per
    tile (16 KiB runs vs 1 KiB), collapsing descriptor count ~16x and
    un-blocking the sync queue (110 us of DMA_DIRECT2D issuance in v3).
  - retrieval row-tiled in pairs: Wv chunk 2j in PE rows 0-63, chunk
    2j+1 in rows 64-127 (Ec duplicated to partitions 64-127 by a tiny
    SBUF->SBUF DMA); 16 -> 8 retrieval pass-slots per tile.
  - S-merge fused into one DVE scalar_tensor_tensor; gvd via
    tensor_tensor_reduce.

DMA floor: (16+8+16.3) MiB / 358 GB/s ~= 118 us/core.
"""

from contextlib import ExitStack

import numpy as np

import concourse.bass as bass
import concourse.tile as tile
from concourse import bacc
from concourse import mybir
from concourse.bass import ts
from concourse.bass_utils import run_bass_kernel_spmd
from concourse.masks import make_identity

F32 = mybir.dt.float32
F16 = mybir.dt.float16
F8 = mybir.dt.float8e4
AX_X = mybir.AxisListType
ALU = mybir.AluOpType
ACTF = mybir.ActivationFunctionType
DR = mybir.MatmulPerfMode.DoubleRow

B = 8
L = 4096
DIM = 2048
NSLOT = 64
NCH = DIM // 128  # 16 dim chunks
TOK = 512  # tokens per tile
NT = L // TOK  # 8 tiles per core
NQ = TOK // 128  # 4 token quarters per tile
NPAIR = NCH // 2  # 8 retrieval chunk pairs
CSCALE = 1024.0  # 2^10 scale of the fp8 correction pass
MPAD = 80  # fp8 DoubleRow weight APs need step%16==0, so pad 65 -> 80 cols
FT = NCH * TOK  # flat free size of one tile per partition


def _build(gate_b: float) -> bass.Bass:
    nc = bacc.Bacc("TRN2", target_bir_lowering=False, debug=False)

    xt = nc.dram_tensor("xt", [128, NT * FT], F16, kind="ExternalInput").ap()
    xlt = nc.dram_tensor("xlt", [128, NT * FT], F8, kind="ExternalInput").ap()
    # xh8 for dim-chunks 0-7 only; chunks 8-15 are converted on ACT
    xht = nc.dram_tensor("xht", [128, NT * FT // 2], F8, kind="ExternalInput").ap()
    GTt = nc.dram_tensor(
        "GTt", [128, NCH * (NSLOT + 1)], F16, kind="ExternalInput"
    ).ap()
    GCt = nc.dram_tensor("GCt", [128, 2 * NCH * MPAD], F8, kind="ExternalInput").ap()
    WvP = nc.dram_tensor("WvP", [128, NPAIR * 128], F16, kind="ExternalInput").ap()
    gv = nc.dram_tensor("gv", [1, NSLOT], F32, kind="ExternalInput").ap()
    outt = nc.dram_tensor("outt", [128, NT * FT], F16, kind="ExternalOutput").ap()

    with tile.TileContext(nc) as tc, ExitStack() as ctx:
        consts = ctx.enter_context(tc.tile_pool(name="consts", bufs=1))
        xpool = ctx.enter_context(tc.tile_pool(name="xpool", bufs=4))
        qpool = ctx.enter_context(tc.tile_pool(name="qpool", bufs=3))
        opool = ctx.enter_context(tc.tile_pool(name="opool", bufs=2))
        work = ctx.enter_context(tc.tile_pool(name="work", bufs=3))
        small = ctx.enter_context(tc.tile_pool(name="small", bufs=3))
        psA = ctx.enter_context(tc.tile_pool(name="psA", bufs=1, space="PSUM"))
        psA2 = ctx.enter_context(tc.tile_pool(name="psA2", bufs=1, space="PSUM"))
        psT = ctx.enter_context(tc.tile_pool(name="psT", bufs=1, space="PSUM"))
        psE = ctx.enter_context(tc.tile_pool(name="psE", bufs=1, space="PSUM"))
        psR = ctx.enter_context(tc.tile_pool(name="psR", bufs=4, space="PSUM"))

        ident = consts.tile([128, 128], F32)
        make_identity(nc, ident)
        ident16 = consts.tile([128, 128], F16)
        nc.scalar.copy(ident16, ident)
        GT_sb = consts.tile([128, NCH, NSLOT + 1], F16)
        nc.sync.dma_start(
            out=GT_sb.rearrange("p c m -> p (c m)"), in_=GTt
        )
        GC_sb = consts.tile([128, 2 * NCH, MPAD], F8)
        nc.sync.dma_start(out=GC_sb.rearrange("p s m -> p (s m)"), in_=GCt)
        # WvP_sb[p<64, j, :]  = -Wv[slot p,  chunk 2j]
        # WvP_sb[p>=64, j, :] = -Wv[slot p-64, chunk 2j+1]
        WvP_sb = consts.tile([128, NPAIR, 128], F16)
        nc.sync.dma_start(out=WvP_sb.rearrange("p j q -> p (j q)"), in_=WvP)
        gv_rep = consts.tile([128, NSLOT], F32)
        nc.sync.dma_start(out=gv_rep, in_=gv.to_broadcast((128, NSLOT)))
        ones16 = consts.tile([NSLOT + 1, 128], F16)
        nc.vector.memset(ones16, 1.0)

        def phase_A_dma(t):
            """prefetch x tile + xl + xh8 (dim-chunks 0-7)."""
            st = {}
            x_sb = xpool.tile([128, NCH, TOK], F16, tag="x_sb")
            nc.sync.dma_start(
                out=x_sb.rearrange("p c k -> p (c k)"), in_=xt[:, ts(t, FT)]
            )
            xq8 = qpool.tile([128, 2 * NCH, TOK], F8, tag="xq8")
            nc.sync.dma_start(
                out=xq8[:, 0:NCH, :].rearrange("p c k -> p (c k)"),
                in_=xlt[:, ts(t, FT)],
            )
            nc.sync.dma_start(
                out=xq8[:, NCH : NCH + NCH // 2, :].rearrange("p c k -> p (c k)"),
                in_=xht[:, ts(t, FT // 2)],
            )
            st["x_sb"] = x_sb
            st["xq8"] = xq8
            return st

        def act_cvt(st):
            """ACT converts dim-chunks 8-15 of xh -> fp8."""
            xq8, x_sb = st["xq8"], st["x_sb"]
            for h in range(2, 4):
                nc.scalar.copy(
                    xq8[:, NCH + 4 * h : NCH + 4 * h + 4, :],
                    x_sb[:, 4 * h : 4 * h + 4, :],
                )

        def S_corr_mms(st, S2_ps):
            """fp8 DoubleRow pass: (xl*2^10)@e4m3(Gh) + e4m3(xh)@(Gl*2^10)."""
            xq8 = st["xq8"]
            for i in range(NCH):
                nc.tensor.matmul(
                    S2_ps[0:MPAD, :],
                    GC_sb[:, 2 * i : 2 * i + 2, :],
                    xq8[:, 2 * i : 2 * i + 2, :],
                    start=(i == 0),
                    stop=(i == NCH - 1),
                    perf_mode=DR,
                    skip_group_check=True,
                )

        def S_merge(st, S_ps, S2_ps):
            """S_sb = main + corr * 2^-10, staged for the transposes."""
            S_c = work.tile([NSLOT + 1, TOK], F32, tag="S_c")
            nc.scalar.activation(
                S_c, S2_ps[0 : NSLOT + 1, :], func=ACTF.Copy, scale=1.0 / CSCALE
            )
            S_sb = work.tile([NSLOT + 1, TOK], F32, tag="S_sb")
            nc.vector.tensor_add(S_sb, S_ps[0 : NSLOT + 1, :], S_c)
            Stok = psT.tile([128, NQ, NSLOT + 1], F32, tag="T")
            for q in range(NQ):
                nc.tensor.transpose(
                    Stok[:, q, :],
                    S_sb[:, ts(q, 128)],
                    ident[0 : NSLOT + 1, 0 : NSLOT + 1],
                )
            st["Stok"] = Stok

        def phase_A_mm(t, st):
            """S matmuls (main fp16 + fp8 corr) -> merge -> transposes."""
            x_sb = st["x_sb"]
            S_ps = psA.tile([128, TOK], F32, tag="A")
            for c in range(NCH):
                nc.tensor.matmul(
                    S_ps[0 : NSLOT + 1, :],
                    GT_sb[:, c, :],
                    x_sb[:, c, :],
                    start=(c == 0),
                    stop=(c == NCH - 1),
                )
            S2_ps = psA2.tile([128, TOK], F32, tag="A2")
            S_corr_mms(st, S2_ps)
            S_merge(st, S_ps, S2_ps)

        def phase_B(t, st):
            """Batched softmax/gate stats; Ec' = -E*r*en, g in row 64."""
            Stok = st["Stok"]
            Etok = [
                small.tile([128, NSLOT], F32, tag=f"Etok{q}", name=f"Etok{q}")
                for q in range(NQ)
            ]
            Ec = small.tile([128, NQ, NSLOT + 1], F32, tag="Ec")
            scr = small.tile([128, NSLOT], F32, tag="scr")
            mb4 = small.tile([128, NQ], F32, tag="mb4")
            sums4 = small.tile([128, NQ], F32, tag="sums4")
            gvd4 = small.tile([128, NQ], F32, tag="gvd4")
            st4 = small.tile([128, 6, NQ], F32, tag="st4")
            mx4, r4, t4, gl4, en4, g4 = (st4[:, i, :] for i in range(6))
            cp4 = small.tile([128, NQ], F32, tag="cp4")
            nc.vector.tensor_reduce(mx4, Stok[:, :, 0:NSLOT], axis=AX_X.X, op=ALU.max)
            nc.vector.tensor_scalar_mul(mb4, mx4, -10.0)
            for q in range(NQ):
                nc.scalar.activation(
                    Etok[q],
                    Stok[:, q, 0:NSLOT],
                    func=ACTF.Exp,
                    bias=mb4[:, q : q + 1],
                    scale=10.0,
                    accum_out=sums4[:, q : q + 1],
                )
            for q in range(NQ):
                nc.vector.tensor_mul(scr, Etok[q], gv_rep)
                nc.vector.tensor_reduce(
                    gvd4[:, q : q + 1], scr, axis=AX_X.X, op=ALU.add
                )
            nc.vector.reciprocal(r4, sums4)
            nc.vector.tensor_mul(t4, gvd4, r4)
            nc.vector.tensor_add(gl4, t4, Stok[:, :, NSLOT])
            nc.scalar.activation(en4, gl4, func=ACTF.Exp, bias=-gate_b, scale=-1.0)
            nc.vector.tensor_scalar_add(g4, en4, 1.0)
            nc.vector.reciprocal(g4, g4)
            nc.vector.tensor_mul(cp4, r4, en4)
            nc.vector.tensor_scalar_mul(cp4, cp4, -1.0)
            for q in range(NQ):
                nc.vector.tensor_scalar_mul(
                    Ec[:, q, 0:NSLOT], Etok[q], cp4[:, q : q + 1]
                )
            for q in range(NQ):
                nc.vector.tensor_copy(Ec[:, q, NSLOT : NSLOT + 1], g4[:, q : q + 1])
            st["Ec"] = Ec

        def phase_C_pre(t, st):
            """Ec -> slot-major (+dup to partitions 64-127), g broadcast.

            Issued BEFORE phase_A_dma(t+3) so the tiny E2-dup DMA lands on
            the sync queue ahead of the 3 MiB bulk prefetch.
            """
            Ec = st["Ec"]
            ET = psE.tile([NSLOT + 1, NQ, 128], F32, tag="E")
            for q in range(NQ):
                nc.tensor.transpose(ET[:, q, :], Ec[:, q, :], ident)
            E_sb = work.tile([NSLOT + 1, NQ, 128], F16, tag="E_sb")
            nc.scalar.copy(E_sb, ET)
            # duplicate slot rows to partitions 64-127 for the row-tiled pair
            # (idle gpsimd SWDGE queue: never queues behind bulk transfers)
            E2 = work.tile([128, NQ, 128], F16, tag="E2")
            nc.gpsimd.dma_start(out=E2[64:128, :, :], in_=E_sb[0:NSLOT, :, :])
            # broadcast the gate row (partition 64) to all partitions via a
            # ones-matmul into the psA2 bank (free between the corr-merge
            # read and the next tile's DR writes)
            g_bc = psA2.tile([128, TOK], F32, tag="A2")
            nc.tensor.matmul(
                g_bc,
                ones16[NSLOT : NSLOT + 1, :],
                E_sb.rearrange("p a b -> p (a b)")[NSLOT : NSLOT + 1, :],
                start=True,
                stop=True,
                skip_group_check=True,
            )
            g_sb = work.tile([128, TOK], F32, tag="g_sb")
            nc.scalar.copy(g_sb, g_bc)
            st["E_sb"], st["E2"], st["g_sb"] = E_sb, E2, g_sb

        def phase_C(t, st, s_next=None):
            """Row-tiled pairs: PSUM = xh + Ec'@WvN ; out = g*PSUM ; one out
            DMA per tile.

            s_next=(t2, st2): interleave tile t2's S matmuls between this
            tile's I/R matmuls so PE stays busy while DVE paces the combine.
            """
            x_sb = st["x_sb"]
            E_flat = st["E_sb"].rearrange("p a b -> p (a b)")  # [65, 512]
            E2_flat = st["E2"].rearrange("p a b -> p (a b)")  # rows 64:128
            g_sb = st["g_sb"]
            if s_next is not None:
                t2, st2 = s_next
                S_ps2 = psA.tile([128, TOK], F32, tag="A")
            o16 = opool.tile([128, NCH, TOK], F16, tag="o16")
            pend = []  # (cA, R_A, cB, R_B) with combine lagging one pair

            def drain_pair():
                cA, R_A, cB, R_B = pend.pop(0)
                nc.vector.tensor_mul(o16[:, cA, :], R_A, g_sb)
                nc.vector.tensor_mul(o16[:, cB, :], R_B, g_sb)
                if cB % 4 == 3:
                    # quarter-tile out-DMA right after its chunks complete:
                    # a whole-tile DMA head-blocked its queue ~20us waiting
                    # on the final DVE mul.
                    q4 = cB // 4
                    nc.scalar.dma_start(
                        out=outt[:, ts(t * 4 + q4, FT // 4)],
                        in_=o16[:, 4 * q4 : 4 * q4 + 4, :].rearrange(
                            "p c k -> p (c k)"
                        ),
                    )

            for j in range(NPAIR):
                cA, cB = 2 * j, 2 * j + 1
                R_A = psR.tile([128, TOK], F32, tag="R")
                R_B = psR.tile([128, TOK], F32, tag="R")
                nc.tensor.matmul(
                    R_A, ident16, x_sb[:, cA, :],
                    start=True, stop=False, skip_group_check=True,
                )
                nc.tensor.matmul(
                    R_B, ident16, x_sb[:, cB, :],
                    start=True, stop=False, skip_group_check=True,
                )
                nc.tensor.matmul(
                    R_A, WvP_sb[0:NSLOT, j, :], E_flat[0:NSLOT, :],
                    start=False, stop=True, skip_group_check=True,
                )
                nc.tensor.matmul(
                    R_B, WvP_sb[NSLOT:128, j, :], E2_flat[NSLOT:128, :],
                    start=False, stop=True, skip_group_check=True,
                )
                if s_next is not None:
                    for c in (cA, cB):
                        nc.tensor.matmul(
                            S_ps2[0 : NSLOT + 1, :],
                            GT_sb[:, c, :],
                            st2["x_sb"][:, c, :],
                            start=(c == 0),
                            stop=(c == NCH - 1),
                            skip_group_check=True,
                        )
                pend.append((cA, R_A, cB, R_B))
                if len(pend) > 1:
                    drain_pair()
            while pend:
                drain_pair()
            if s_next is not None:
                t2, st2 = s_next
                S2_ps2 = psA2.tile([128, TOK], F32, tag="A2")
                S_corr_mms(st2, S2_ps2)
                S_merge(st2, S_ps2, S2_ps2)

        # software pipeline, 3 tiles deep (see v3 notes): x-DMA 3 tiles
        # ahead; tile t+2's S matmuls interleave into tile t's combine;
        # tile t+2's stats issue AFTER phase_C(t).
        states = {}
        states[0] = phase_A_dma(0)
        states[1] = phase_A_dma(1)
        states[2] = phase_A_dma(2)
        act_cvt(states[0])
        act_cvt(states[1])
        act_cvt(states[2])
        phase_A_mm(0, states[0])
        phase_A_mm(1, states[1])
        phase_B(0, states[0])
        phase_B(1, states[1])
        phase_C_pre(0, states[0])
        for t in range(NT):
            if t + 3 < NT:
                states[t + 3] = phase_A_dma(t + 3)
            phase_C(
                t,
                states[t],
                s_next=(t + 2, states[t + 2]) if t + 2 < NT else None,
            )
            if t + 2 < NT:
                phase_B(t + 2, states[t + 2])
            if t + 1 < NT:
                phase_C_pre(t + 1, states[t + 1])
            if t + 3 < NT:
                act_cvt(states[t + 3])
            del states[t]

    nc.compile()
    return nc


def _fold_weights(memory, key_w, value_w, gate_w):
    """Fold module weights; returns un-packed (GT, GC, WvN, gv) as in v3."""
    mem = memory.astype(np.float64)
    Ws = (mem @ key_w.astype(np.float64)).astype(np.float32)  # [64, 2048]
    Wv = (mem @ value_w.astype(np.float64).T).astype(np.float32)  # [64, 2048]
    gx = np.asarray(gate_w[0, :DIM], dtype=np.float32)
    gvv = (Wv.astype(np.float64) @ gate_w[0, DIM:].astype(np.float64)).astype(
        np.float32
    )
    G = np.concatenate([Ws, gx[None, :]], axis=0)  # [65, 2048]; gate row last
    WvN = np.ascontiguousarray(-Wv).astype(np.float16)  # [64, 2048]
    F8NP = mybir.dt.np(F8)
    Gh = G.astype(np.float16)  # [65, 2048]
    Gh8 = Gh.astype(F8NP)
    Gl10 = ((G - Gh.astype(np.float32)) * CSCALE).astype(F8NP)
    GT = np.ascontiguousarray(Gh.T)  # [2048, 65] fp16
    GC = np.zeros((2 * DIM, MPAD), dtype=F8NP)
    GC[:DIM, : NSLOT + 1] = Gh8.T
    GC[DIM:, : NSLOT + 1] = Gl10.T
    return GT, GC, WvN, gvv.reshape(1, NSLOT)


def _pack_weights(GT, GC, WvN):
    """Device layouts: [128 partitions, contiguous free]."""
    # GT [2048, 65] -> [128, NCH*65]; row d = c*128+p
    GTt = np.ascontiguousarray(
        GT.reshape(NCH, 128, NSLOT + 1).transpose(1, 0, 2).reshape(128, -1)
    )
    # GC [4096, 80] -> [128, 32*80]
    GCt = np.ascontiguousarray(
        GC.reshape(2 * NCH, 128, MPAD).transpose(1, 0, 2).reshape(128, -1)
    )
    # WvN [64, 2048] -> WvP [128, NPAIR*128]
    Wv3 = WvN.reshape(NSLOT, NCH, 128)
    WvP = np.concatenate([Wv3[:, 0::2, :], Wv3[:, 1::2, :]], axis=0)
    WvP = np.ascontiguousarray(WvP.reshape(128, -1))
    return GTt, GCt, WvP


def _pack_x(xb):
    """x [L, DIM] f32 -> (xt f16, xlt f8, xht f8 [dim-chunks 0-7])."""
    F8NP = mybir.dt.np(F8)
    xT = np.ascontiguousarray(xb.T)  # [2048, 4096]
    xh = xT.astype(np.float16)
    xl8 = ((xT - xh.astype(np.float32)) * CSCALE).astype(F8NP)
    # [d= c*128+p, tok= t*512+k] -> [p, t, c, k]
    xh4 = xh.reshape(NCH, 128, NT, TOK)
    xtp = np.ascontiguousarray(xh4.transpose(1, 2, 0, 3).reshape(128, -1))
    xlp = np.ascontiguousarray(
        xl8.reshape(NCH, 128, NT, TOK).transpose(1, 2, 0, 3).reshape(128, -1)
    )
    xhp = np.ascontiguousarray(
        xh4[: NCH // 2].astype(F8NP).transpose(1, 2, 0, 3).reshape(128, -1)
    )
    return xtp, xlp, xhp


def _unpack_out(o):
    """outt [128, NT*FT] f16 -> out [L, DIM] f32."""
    # [p, t, c, k] -> [d= c*128+p, tok= t*512+k]
    oT = o.reshape(128, NT, NCH, TOK).transpose(2, 0, 1, 3).reshape(DIM, L)
    return oT.T.astype(np.float32)


def kernel(x, memory, key_w, value_w, gate_w, gate_b, _trace=False, _tmpdir=None):
    x = np.asarray(x, dtype=np.float32)
    GT, GC, WvN, gvv = _fold_weights(
        np.asarray(memory, np.float32),
        np.asarray(key_w, np.float32),
        np.asarray(value_w, np.float32),
        np.asarray(gate_w, np.float32),
    )
    GTt, GCt, WvP = _pack_weights(GT, GC, WvN)
    nc = _build(float(np.asarray(gate_b).reshape(-1)[0]))
    in_maps = []
    for b in range(B):
        xtp, xlp, xhp = _pack_x(x[b])
        in_maps.append(
            {"xt": xtp, "xlt": xlp, "xht": xhp,
             "GTt": GTt, "GCt": GCt, "WvP": WvP, "gv": gvv}
        )
    res = run_bass_kernel_spmd(
        nc, in_maps, list(range(B)), trace=_trace, tmpdir=_tmpdir
    )
    out = np.stack(
        [_unpack_out(res.results[b]["outt"]) for b in range(B)], axis=0
    )
    if _trace:
        return out, res
    return out


def sim_core0(inputs, expected):
    """CoreSim check of core 0 against expected[0]; returns maxabs err."""
    from concourse.bass_interp import CoreSim

    GT, GC, WvN, gvv = _fold_weights(
        inputs["memory"], inputs["key_w"], inputs["value_w"], inputs["gate_w"]
    )
    GTt, GCt, WvP = _pack_weights(GT, GC, WvN)
    nc = _build(float(inputs["gate_b"][0]))
    sim = CoreSim(nc)
    xtp, xlp, xhp = _pack_x(inputs["x"][0])
    sim.tensor("xt")[:] = xtp
    sim.tensor("xlt")[:] = xlp
    sim.tensor("xht")[:] = xhp
    sim.tensor("GTt")[:] = GTt
    sim.tensor("GCt")[:] = GCt
    sim.tensor("WvP")[:] = WvP
    sim.tensor("gv")[:] = gvv
    sim.simulate()
    got = _unpack_out(np.asarray(sim.tensor("outt")))
    return np.abs(got - expected[0]).max()


# revision 29
# speedup vs baseline: 1.0188x; 1.0188x over previous
"""MemoryBank kernel v4 for 8x TRN2 NeuronCores (SPMD, batch-parallel).

Same folded algebra as v3 (split-precision scores, exp-based gate fold):

    x  = xh (fp16, DMA'd) + xl (e4m3 * 2^-10, DMA'd)
    S  = xh@Gh  (fp16 full-rate)
       + 2^-10 * [ (xl*2^10)@e4m3(Gh) + e4m3(xh)@(Gl*2^10) ]   (fp8 DoubleRow)
    en = exp(-gate_logit); g = 1/(1+en); Ec' = -E*r*en
    PSUM = xh + Ec'@WvN   (identity-pass + retrieval on PE)
    out  = g * PSUM       (single DVE op per element, fp16 out)

v4 changes vs v3 (227 us):
  - xh8 derived ON-CHIP (8 chunks gpsimd + 8 chunks ACT converting copies)
    instead of DMA'd: HBM traffic 48.8 -> 40.3 MiB.
  - pre-tiled DRAM layouts: every stream is [128 part, contiguous] per
    tile (16 KiB runs vs 1 KiB), collapsing descriptor count ~16x and
    un-blocking the sync queue (110 us of DMA_DIRECT2D issuance in v3).
  - retrieval row-tiled in pairs: Wv chunk 2j in PE rows 0-63, chunk
    2j+1 in rows 64-127 (Ec duplicated to partitions 64-127 by a tiny
    SBUF->SBUF DMA); 16 -> 8 retrieval pass-slots per tile.
  - S-merge fused into one DVE scalar_tensor_tensor; gvd via
    tensor_tensor_reduce.

DMA floor: (16+8+16.3) MiB / 358 GB/s ~= 118 us/core.
"""

from contextlib import ExitStack

import numpy as np

import concourse.bass as bass
import concourse.tile as tile
from concourse import bacc
from concourse import mybir
from concourse.bass import ts
from concourse.bass_utils import run_bass_kernel_spmd
from concourse.masks import make_identity

F32 = mybir.dt.float32
F16 = mybir.dt.float16
F8 = mybir.dt.float8e4
AX_X = mybir.AxisListType
ALU = mybir.AluOpType
ACTF = mybir.ActivationFunctionType
DR = mybir.MatmulPerfMode.DoubleRow

B = 8
L = 4096
DIM = 2048
NSLOT = 64
NCH = DIM // 128  # 16 dim chunks
TOK = 512  # tokens per tile
NT = L // TOK  # 8 tiles per core
NQ = TOK // 128  # 4 token quarters per tile
NPAIR = NCH // 2  # 8 retrieval chunk pairs
CSCALE = 1024.0  # 2^10 scale of the fp8 correction pass
MPAD = 80  # fp8 DoubleRow weight APs need step%16==0, so pad 65 -> 80 cols
FT = NCH * TOK  # flat free size of one tile per partition


def _build(gate_b: float) -> bass.Bass:
    nc = bacc.Bacc("TRN2", target_bir_lowering=False, debug=False)

    xt = nc.dram_tensor("xt", [128, NT * FT], F16, kind="ExternalInput").ap()
    xlt = nc.dram_tensor("xlt", [128, NT * FT], F8, kind="ExternalInput").ap()
    # xh8 for dim-chunks 0-7 only; chunks 8-15 are converted on ACT
    xht = nc.dram_tensor("xht", [128, NT * FT // 2], F8, kind="ExternalInput").ap()
    GTt = nc.dram_tensor(
        "GTt", [128, NCH * (NSLOT + 1)], F16, kind="ExternalInput"
    ).ap()
    GCt = nc.dram_tensor("GCt", [128, 2 * NCH * MPAD], F8, kind="ExternalInput").ap()
    WvP = nc.dram_tensor("WvP", [128, NPAIR * 128], F16, kind="ExternalInput").ap()
    gv = nc.dram_tensor("gv", [1, NSLOT], F32, kind="ExternalInput").ap()
    outt = nc.dram_tensor("outt", [128, NT * FT], F16, kind="ExternalOutput").ap()

    with tile.TileContext(nc) as tc, ExitStack() as ctx:
        consts = ctx.enter_context(tc.tile_pool(name="consts", bufs=1))
        xpool = ctx.enter_context(tc.tile_pool(name="xpool", bufs=4))
        qpool = ctx.enter_context(tc.tile_pool(name="qpool", bufs=3))
        opool = ctx.enter_context(tc.tile_pool(name="opool", bufs=2))
        work = ctx.enter_context(tc.tile_pool(name="work", bufs=3))
        small = ctx.enter_context(tc.tile_pool(name="small", bufs=3))
        psA = ctx.enter_context(tc.tile_pool(name="psA", bufs=1, space="PSUM"))
        psA2 = ctx.enter_context(tc.tile_pool(name="psA2", bufs=1, space="PSUM"))
        psT = ctx.enter_context(tc.tile_pool(name="psT", bufs=1, space="PSUM"))
        psE = ctx.enter_context(tc.tile_pool(name="psE", bufs=1, space="PSUM"))
        psR = ctx.enter_context(tc.tile_pool(name="psR", bufs=4, space="PSUM"))

        ident = consts.tile([128, 128], F32)
        make_identity(nc, ident)
        ident16 = consts.tile([128, 128], F16)
        nc.scalar.copy(ident16, ident)
        GT_sb = consts.tile([128, NCH, NSLOT + 1], F16)
        nc.sync.dma_start(
            out=GT_sb.rearrange("p c m -> p (c m)"), in_=GTt
        )
        GC_sb = consts.tile([128, 2 * NCH, MPAD], F8)
        nc.sync.dma_start(out=GC_sb.rearrange("p s m -> p (s m)"), in_=GCt)
        # WvP_sb[p<64, j, :]  = -Wv[slot p,  chunk 2j]
        # WvP_sb[p>=64, j, :] = -Wv[slot p-64, chunk 2j+1]
        WvP_sb = consts.tile([128, NPAIR, 128], F16)
        nc.sync.dma_start(out=WvP_sb.rearrange("p j q -> p (j q)"), in_=WvP)
        gv_rep = consts.tile([128, NSLOT], F32)
        nc.sync.dma_start(out=gv_rep, in_=gv.to_broadcast((128, NSLOT)))
        ones16 = consts.tile([NSLOT + 1, 128], F16)
        nc.vector.memset(ones16, 1.0)

        def phase_A_dma(t):
            """prefetch x tile + xl + xh8 (dim-chunks 0-7)."""
            st = {}
            x_sb = xpool.tile([128, NCH, TOK], F16, tag="x_sb")
            nc.sync.dma_start(
                out=x_sb.rearrange("p c k -> p (c k)"), in_=xt[:, ts(t, FT)]
            )
            xq8 = qpool.tile([128, 2 * NCH, TOK], F8, tag="xq8")
            nc.sync.dma_start(
                out=xq8[:, 0:NCH, :].rearrange("p c k -> p (c k)"),
                in_=xlt[:, ts(t, FT)],
            )
            nc.sync.dma_start(
                out=xq8[:, NCH : NCH + NCH // 2, :].rearrange("p c k -> p (c k)"),
                in_=xht[:, ts(t, FT // 2)],
            )
            st["x_sb"] = x_sb
            st["xq8"] = xq8
            return st

        def act_cvt(st):
            """ACT converts dim-chunks 8-15 of xh -> fp8."""
            xq8, x_sb = st["xq8"], st["x_sb"]
            for h in range(2, 4):
                nc.scalar.copy(
                    xq8[:, NCH + 4 * h : NCH + 4 * h + 4, :],
                    x_sb[:, 4 * h : 4 * h + 4, :],
                )

        def S_corr_mms(st, S2_ps):
            """fp8 DoubleRow pass: (xl*2^10)@e4m3(Gh) + e4m3(xh)@(Gl*2^10)."""
            xq8 = st["xq8"]
            for i in range(NCH):
                nc.tensor.matmul(
                    S2_ps[0:MPAD, :],
                    GC_sb[:, 2 * i : 2 * i + 2, :],
                    xq8[:, 2 * i : 2 * i + 2, :],
                    start=(i == 0),
                    stop=(i == NCH - 1),
                    perf_mode=DR,
                    skip_group_check=True,
                )

        def S_merge(st, S_ps, S2_ps):
            """S_sb = main + corr * 2^-10, staged for the transposes."""
            S_c = work.tile([NSLOT + 1, TOK], F32, tag="S_c")
            nc.scalar.activation(
                S_c, S2_ps[0 : NSLOT + 1, :], func=ACTF.Copy, scale=1.0 / CSCALE
            )
            S_sb = work.tile([NSLOT + 1, TOK], F32, tag="S_sb")
            nc.vector.tensor_add(S_sb, S_ps[0 : NSLOT + 1, :], S_c)
            Stok = psT.tile([128, NQ, NSLOT + 1], F32, tag="T")
            for q in range(NQ):
                nc.tensor.transpose(
                    Stok[:, q, :],
                    S_sb[:, ts(q, 128)],
                    ident[0 : NSLOT + 1, 0 : NSLOT + 1],
                )
            st["Stok"] = Stok

        def phase_A_mm(t, st):
            """S matmuls (main fp16 + fp8 corr) -> merge -> transposes."""
            x_sb = st["x_sb"]
            S_ps = psA.tile([128, TOK], F32, tag="A")
            for c in range(NCH):
                nc.tensor.matmul(
                    S_ps[0 : NSLOT + 1, :],
                    GT_sb[:, c, :],
                    x_sb[:, c, :],
                    start=(c == 0),
                    stop=(c == NCH - 1),
                )
            S2_ps = psA2.tile([128, TOK], F32, tag="A2")
            S_corr_mms(st, S2_ps)
            S_merge(st, S_ps, S2_ps)

        def phase_B(t, st):
            """Batched softmax/gate stats; Ec' = -E*r*en, g in row 64."""
            Stok = st["Stok"]
            Etok = [
                small.tile([128, NSLOT], F32, tag=f"Etok{q}", name=f"Etok{q}")
                for q in range(NQ)
            ]
            Ec = small.tile([128, NQ, NSLOT + 1], F32, tag="Ec")
            scr = small.tile([128, NSLOT], F32, tag="scr")
            mb4 = small.tile([128, NQ], F32, tag="mb4")
            sums4 = small.tile([128, NQ], F32, tag="sums4")
            gvd4 = small.tile([128, NQ], F32, tag="gvd4")
            st4 = small.tile([128, 6, NQ], F32, tag="st4")
            mx4, r4, t4, gl4, en4, g4 = (st4[:, i, :] for i in range(6))
            cp4 = small.tile([128, NQ], F32, tag="cp4")
            nc.vector.tensor_reduce(mx4, Stok[:, :, 0:NSLOT], axis=AX_X.X, op=ALU.max)
            nc.vector.tensor_scalar_mul(mb4, mx4, -10.0)
            for q in range(NQ):
                nc.scalar.activation(
                    Etok[q],
                    Stok[:, q, 0:NSLOT],
                    func=ACTF.Exp,
                    bias=mb4[:, q : q + 1],
                    scale=10.0,
                    accum_out=sums4[:, q : q + 1],
                )
            for q in range(NQ):
                nc.vector.tensor_mul(scr, Etok[q], gv_rep)
                nc.vector.tensor_reduce(
                    gvd4[:, q : q + 1], scr, axis=AX_X.X, op=ALU.add
                )
            nc.vector.reciprocal(r4, sums4)
            nc.vector.tensor_mul(t4, gvd4, r4)
            nc.vector.tensor_add(gl4, t4, Stok[:, :, NSLOT])
            nc.scalar.activation(en4, gl4, func=ACTF.Exp, bias=-gate_b, scale=-1.0)
            nc.vector.tensor_scalar_add(g4, en4, 1.0)
            nc.vector.reciprocal(g4, g4)
            nc.vector.tensor_mul(cp4, r4, en4)
            nc.vector.tensor_scalar_mul(cp4, cp4, -1.0)
            for q in range(NQ):
                nc.vector.tensor_scalar_mul(
                    Ec[:, q, 0:NSLOT], Etok[q], cp4[:, q : q + 1]
                )
            for q in range(NQ):
                nc.vector.tensor_copy(Ec[:, q, NSLOT : NSLOT + 1], g4[:, q : q + 1])
            st["Ec"] = Ec

        def phase_C_pre(t, st):
            """Ec -> slot-major (+dup to partitions 64-127), g broadcast.

            Issued BEFORE phase_A_dma(t+3) so the tiny E2-dup DMA lands on
            the sync queue ahead of the 3 MiB bulk prefetch.
            """
            Ec = st["Ec"]
            ET = psE.tile([NSLOT + 1, NQ, 128], F32, tag="E")
            for q in range(NQ):
                nc.tensor.transpose(ET[:, q, :], Ec[:, q, :], ident)
            E_sb = work.tile([NSLOT + 1, NQ, 128], F16, tag="E_sb")
            nc.scalar.copy(E_sb, ET)
            # duplicate slot rows to partitions 64-127 for the row-tiled pair
            # (idle gpsimd SWDGE queue: never queues behind bulk transfers)
            E2 = work.tile([128, NQ, 128], F16, tag="E2")
            nc.gpsimd.dma_start(out=E2[64:128, :, :], in_=E_sb[0:NSLOT, :, :])
            # broadcast the gate row (partition 64) to all partitions via a
            # ones-matmul into the psA2 bank (free between the corr-merge
            # read and the next tile's DR writes)
            g_bc = psA2.tile([128, TOK], F32, tag="A2")
            nc.tensor.matmul(
                g_bc,
                ones16[NSLOT : NSLOT + 1, :],
                E_sb.rearrange("p a b -> p (a b)")[NSLOT : NSLOT + 1, :],
                start=True,
                stop=True,
                skip_group_check=True,
            )
            g_sb = work.tile([128, TOK], F32, tag="g_sb")
            nc.scalar.copy(g_sb, g_bc)
            st["E_sb"], st["E2"], st["g_sb"] = E_sb, E2, g_sb

        def phase_C(t, st, s_next=None):
            """Row-tiled pairs: PSUM = xh + Ec'@WvN ; out = g*PSUM ; one out
            DMA per tile.

            s_next=(t2, st2): interleave tile t2's S matmuls between this
            tile's I/R matmuls so PE stays busy while DVE paces the combine.
            """
            x_sb = st["x_sb"]
            E_flat = st["E_sb"].rearrange("p a b -> p (a b)")  # [65, 512]
            E2_flat = st["E2"].rearrange("p a b -> p (a b)")  # rows 64:128
            g_sb = st["g_sb"]
            if s_next is not None:
                t2, st2 = s_next
                S_ps2 = psA.tile([128, TOK], F32, tag="A")
                st["S_ps2"] = S_ps2
            o16 = opool.tile([128, NCH, TOK], F16, tag="o16")
            pend = []  # (cA, R_A, cB, R_B) with combine lagging one pair

            def drain_pair():
                cA, R_A, cB, R_B = pend.pop(0)
                nc.vector.tensor_mul(o16[:, cA, :], R_A, g_sb)
                nc.vector.tensor_mul(o16[:, cB, :], R_B, g_sb)
                if cB % 4 == 3:
                    # quarter-tile out-DMA right after its chunks complete:
                    # a whole-tile DMA head-blocked its queue ~20us waiting
                    # on the final DVE mul.
                    q4 = cB // 4
                    nc.scalar.dma_start(
                        out=outt[:, ts(t * 4 + q4, FT // 4)],
                        in_=o16[:, 4 * q4 : 4 * q4 + 4, :].rearrange(
                            "p c k -> p (c k)"
                        ),
                    )

            for j in range(NPAIR):
                cA, cB = 2 * j, 2 * j + 1
                R_A = psR.tile([128, TOK], F32, tag="R")
                R_B = psR.tile([128, TOK], F32, tag="R")
                nc.tensor.matmul(
                    R_A, ident16, x_sb[:, cA, :],
                    start=True, stop=False, skip_group_check=True,
                )
                nc.tensor.matmul(
                    R_B, ident16, x_sb[:, cB, :],
                    start=True, stop=False, skip_group_check=True,
                )
                nc.tensor.matmul(
                    R_A, WvP_sb[0:NSLOT, j, :], E_flat[0:NSLOT, :],
                    start=False, stop=True, skip_group_check=True,
                )
                nc.tensor.matmul(
                    R_B, WvP_sb[NSLOT:128, j, :], E2_flat[NSLOT:128, :],
                    start=False, stop=True, skip_group_check=True,
                )
                if s_next is not None:
                    for c in (cA, cB):
                        nc.tensor.matmul(
                            S_ps2[0 : NSLOT + 1, :],
                            GT_sb[:, c, :],
                            st2["x_sb"][:, c, :],
                            start=(c == 0),
                            stop=(c == NCH - 1),
                            skip_group_check=True,
                        )
                pend.append((cA, R_A, cB, R_B))
                if len(pend) > 1:
                    drain_pair()
            while pend:
                drain_pair()

        def phase_C_tail(t, st, s_next):
            """s_next's DR corr + merge + Stok transposes (PE-queue tail)."""
            if s_next is not None:
                t2, st2 = s_next
                S2_ps2 = psA2.tile([128, TOK], F32, tag="A2")
                S_corr_mms(st2, S2_ps2)
                S_merge(st2, st["S_ps2"], S2_ps2)

        # software pipeline, 3 tiles deep (see v3 notes): x-DMA 3 tiles
        # ahead; tile t+2's S matmuls interleave into tile t's combine;
        # tile t+2's stats issue AFTER phase_C(t).
        states = {}
        states[0] = phase_A_dma(0)
        states[1] = phase_A_dma(1)
        states[2] = phase_A_dma(2)
        act_cvt(states[0])
        act_cvt(states[1])
        act_cvt(states[2])
        phase_A_mm(0, states[0])
        phase_A_mm(1, states[1])
        phase_B(0, states[0])
        phase_B(1, states[1])
        phase_C_pre(0, states[0])
        for t in range(NT):
            if t + 3 < NT:
                states[t + 3] = phase_A_dma(t + 3)
            s_next = (t + 2, states[t + 2]) if t + 2 < NT else None
            phase_C(t, states[t], s_next=s_next)
            if t + 1 < NT:
                phase_C_pre(t + 1, states[t + 1])
            phase_C_tail(t, states[t], s_next)
            if t + 2 < NT:
                phase_B(t + 2, states[t + 2])
            if t + 3 < NT:
                act_cvt(states[t + 3])
            del states[t]

    nc.compile()
    return nc


def _fold_weights(memory, key_w, value_w, gate_w):
    """Fold module weights; returns un-packed (GT, GC, WvN, gv) as in v3."""
    mem = memory.astype(np.float64)
    Ws = (mem @ key_w.astype(np.float64)).astype(np.float32)  # [64, 2048]
    Wv = (mem @ value_w.astype(np.float64).T).astype(np.float32)  # [64, 2048]
    gx = np.asarray(gate_w[0, :DIM], dtype=np.float32)
    gvv = (Wv.astype(np.float64) @ gate_w[0, DIM:].astype(np.float64)).astype(
        np.float32
    )
    G = np.concatenate([Ws, gx[None, :]], axis=0)  # [65, 2048]; gate row last
    WvN = np.ascontiguousarray(-Wv).astype(np.float16)  # [64, 2048]
    F8NP = mybir.dt.np(F8)
    Gh = G.astype(np.float16)  # [65, 2048]
    Gh8 = Gh.astype(F8NP)
    Gl10 = ((G - Gh.astype(np.float32)) * CSCALE).astype(F8NP)
    GT = np.ascontiguousarray(Gh.T)  # [2048, 65] fp16
    GC = np.zeros((2 * DIM, MPAD), dtype=F8NP)
    GC[:DIM, : NSLOT + 1] = Gh8.T
    GC[DIM:, : NSLOT + 1] = Gl10.T
    return GT, GC, WvN, gvv.reshape(1, NSLOT)


def _pack_weights(GT, GC, WvN):
    """Device layouts: [128 partitions, contiguous free]."""
    # GT [2048, 65] -> [128, NCH*65]; row d = c*128+p
    GTt = np.ascontiguousarray(
        GT.reshape(NCH, 128, NSLOT + 1).transpose(1, 0, 2).reshape(128, -1)
    )
    # GC [4096, 80] -> [128, 32*80]
    GCt = np.ascontiguousarray(
        GC.reshape(2 * NCH, 128, MPAD).transpose(1, 0, 2).reshape(128, -1)
    )
    # WvN [64, 2048] -> WvP [128, NPAIR*128]
    Wv3 = WvN.reshape(NSLOT, NCH, 128)
    WvP = np.concatenate([Wv3[:, 0::2, :], Wv3[:, 1::2, :]], axis=0)
    WvP = np.ascontiguousarray(WvP.reshape(128, -1))
    return GTt, GCt, WvP


def _pack_x(xb):
    """x [L, DIM] f32 -> (xt f16, xlt f8, xht f8 [dim-chunks 0-7])."""
    F8NP = mybir.dt.np(F8)
    xT = np.ascontiguousarray(xb.T)  # [2048, 4096]
    xh = xT.astype(np.float16)
    xl8 = ((xT - xh.astype(np.float32)) * CSCALE).astype(F8NP)
    # [d= c*128+p, tok= t*512+k] -> [p, t, c, k]
    xh4 = xh.reshape(NCH, 128, NT, TOK)
    xtp = np.ascontiguousarray(xh4.transpose(1, 2, 0, 3).reshape(128, -1))
    xlp = np.ascontiguousarray(
        xl8.reshape(NCH, 128, NT, TOK).transpose(1, 2, 0, 3).reshape(128, -1)
    )
    xhp = np.ascontiguousarray(
        xh4[: NCH // 2].astype(F8NP).transpose(1, 2, 0, 3).reshape(128, -1)
    )
    return xtp, xlp, xhp


def _unpack_out(o):
    """outt [128, NT*FT] f16 -> out [L, DIM] f32."""
    # [p, t, c, k] -> [d= c*128+p, tok= t*512+k]
    oT = o.reshape(128, NT, NCH, TOK).transpose(2, 0, 1, 3).reshape(DIM, L)
    return oT.T.astype(np.float32)


def kernel(x, memory, key_w, value_w, gate_w, gate_b, _trace=False, _tmpdir=None):
    x = np.asarray(x, dtype=np.float32)
    GT, GC, WvN, gvv = _fold_weights(
        np.asarray(memory, np.float32),
        np.asarray(key_w, np.float32),
        np.asarray(value_w, np.float32),
        np.asarray(gate_w, np.float32),
    )
    GTt, GCt, WvP = _pack_weights(GT, GC, WvN)
    nc = _build(float(np.asarray(gate_b).reshape(-1)[0]))
    in_maps = []
    for b in range(B):
        xtp, xlp, xhp = _pack_x(x[b])
        in_maps.append(
            {"xt": xtp, "xlt": xlp, "xht": xhp,
             "GTt": GTt, "GCt": GCt, "WvP": WvP, "gv": gvv}
        )
    res = run_bass_kernel_spmd(
        nc, in_maps, list(range(B)), trace=_trace, tmpdir=_tmpdir
    )
    out = np.stack(
        [_unpack_out(res.results[b]["outt"]) for b in range(B)], axis=0
    )
    if _trace:
        return out, res
    return out


def sim_core0(inputs, expected):
    """CoreSim check of core 0 against expected[0]; returns maxabs err."""
    from concourse.bass_interp import CoreSim

    GT, GC, WvN, gvv = _fold_weights(
        inputs["memory"], inputs["key_w"], inputs["value_w"], inputs["gate_w"]
    )
    GTt, GCt, WvP = _pack_weights(GT, GC, WvN)
    nc = _build(float(inputs["gate_b"][0]))
    sim = CoreSim(nc)
    xtp, xlp, xhp = _pack_x(inputs["x"][0])
    sim.tensor("xt")[:] = xtp
    sim.tensor("xlt")[:] = xlp
    sim.tensor("xht")[:] = xhp
    sim.tensor("GTt")[:] = GTt
    sim.tensor("GCt")[:] = GCt
    sim.tensor("WvP")[:] = WvP
    sim.tensor("gv")[:] = gvv
    sim.simulate()
    got = _unpack_out(np.asarray(sim.tensor("outt")))
    return np.abs(got - expected[0]).max()


# revision 31
# speedup vs baseline: 1.0516x; 1.0322x over previous
"""MemoryBank kernel v4 for 8x TRN2 NeuronCores (SPMD, batch-parallel).

Same folded algebra as v3 (split-precision scores, exp-based gate fold):

    x  = xh (fp16, DMA'd) + xl (e4m3 * 2^-10, DMA'd)
    S  = xh@Gh  (fp16 full-rate)
       + 2^-10 * [ (xl*2^10)@e4m3(Gh) + e4m3(xh)@(Gl*2^10) ]   (fp8 DoubleRow)
    en = exp(-gate_logit); g = 1/(1+en); Ec' = -E*r*en
    PSUM = xh + Ec'@WvN   (identity-pass + retrieval on PE)
    out  = g * PSUM       (single DVE op per element, fp16 out)

v4 changes vs v3 (227 us):
  - xh8 derived ON-CHIP (8 chunks gpsimd + 8 chunks ACT converting copies)
    instead of DMA'd: HBM traffic 48.8 -> 40.3 MiB.
  - pre-tiled DRAM layouts: every stream is [128 part, contiguous] per
    tile (16 KiB runs vs 1 KiB), collapsing descriptor count ~16x and
    un-blocking the sync queue (110 us of DMA_DIRECT2D issuance in v3).
  - retrieval row-tiled in pairs: Wv chunk 2j in PE rows 0-63, chunk
    2j+1 in rows 64-127 (Ec duplicated to partitions 64-127 by a tiny
    SBUF->SBUF DMA); 16 -> 8 retrieval pass-slots per tile.
  - S-merge fused into one DVE scalar_tensor_tensor; gvd via
    tensor_tensor_reduce.

DMA floor: (16+8+16.3) MiB / 358 GB/s ~= 118 us/core.
"""

from contextlib import ExitStack

import numpy as np

import concourse.bass as bass
import concourse.tile as tile
from concourse import bacc
from concourse import mybir
from concourse.bass import ts
from concourse.bass_utils import run_bass_kernel_spmd
from concourse.masks import make_identity

F32 = mybir.dt.float32
F16 = mybir.dt.float16
F8 = mybir.dt.float8e4
AX_X = mybir.AxisListType
ALU = mybir.AluOpType
ACTF = mybir.ActivationFunctionType
DR = mybir.MatmulPerfMode.DoubleRow

B = 8
L = 4096
DIM = 2048
NSLOT = 64
NCH = DIM // 128  # 16 dim chunks
TOK = 512  # tokens per tile
NT = L // TOK  # 8 tiles per core
NQ = TOK // 128  # 4 token quarters per tile
NPAIR = NCH // 2  # 8 retrieval chunk pairs
CSCALE = 1024.0  # 2^10 scale of the fp8 correction pass
MPAD = 80  # fp8 DoubleRow weight APs need step%16==0, so pad 65 -> 80 cols
FT = NCH * TOK  # flat free size of one tile per partition


def _build(gate_b: float) -> bass.Bass:
    nc = bacc.Bacc("TRN2", target_bir_lowering=False, debug=False)

    xt = nc.dram_tensor("xt", [128, NT * FT], F16, kind="ExternalInput").ap()
    xlt = nc.dram_tensor("xlt", [128, NT * FT], F8, kind="ExternalInput").ap()
    # xh8 for dim-chunks 0-7 only; chunks 8-15 are converted on ACT
    xht = nc.dram_tensor("xht", [128, NT * FT // 2], F8, kind="ExternalInput").ap()
    GTt = nc.dram_tensor(
        "GTt", [128, NCH * (NSLOT + 1)], F16, kind="ExternalInput"
    ).ap()
    GCt = nc.dram_tensor("GCt", [128, 2 * NCH * MPAD], F8, kind="ExternalInput").ap()
    WvP = nc.dram_tensor("WvP", [128, NPAIR * 128], F16, kind="ExternalInput").ap()
    gv = nc.dram_tensor("gv", [1, NSLOT], F32, kind="ExternalInput").ap()
    outt = nc.dram_tensor("outt", [128, NT * FT], F16, kind="ExternalOutput").ap()

    with tile.TileContext(nc) as tc, ExitStack() as ctx:
        consts = ctx.enter_context(tc.tile_pool(name="consts", bufs=1))
        xpool = ctx.enter_context(tc.tile_pool(name="xpool", bufs=5))
        qpool = ctx.enter_context(tc.tile_pool(name="qpool", bufs=3))
        opool = ctx.enter_context(tc.tile_pool(name="opool", bufs=2))
        work = ctx.enter_context(tc.tile_pool(name="work", bufs=3))
        small = ctx.enter_context(tc.tile_pool(name="small", bufs=3))
        psA = ctx.enter_context(tc.tile_pool(name="psA", bufs=1, space="PSUM"))
        psA2 = ctx.enter_context(tc.tile_pool(name="psA2", bufs=1, space="PSUM"))
        psT = ctx.enter_context(tc.tile_pool(name="psT", bufs=1, space="PSUM"))
        psE = ctx.enter_context(tc.tile_pool(name="psE", bufs=1, space="PSUM"))
        psR = ctx.enter_context(tc.tile_pool(name="psR", bufs=4, space="PSUM"))

        ident = consts.tile([128, 128], F32)
        make_identity(nc, ident)
        ident16 = consts.tile([128, 128], F16)
        nc.scalar.copy(ident16, ident)
        GT_sb = consts.tile([128, NCH, NSLOT + 1], F16)
        nc.sync.dma_start(
            out=GT_sb.rearrange("p c m -> p (c m)"), in_=GTt
        )
        GC_sb = consts.tile([128, 2 * NCH, MPAD], F8)
        nc.sync.dma_start(out=GC_sb.rearrange("p s m -> p (s m)"), in_=GCt)
        # WvP_sb[p<64, j, :]  = -Wv[slot p,  chunk 2j]
        # WvP_sb[p>=64, j, :] = -Wv[slot p-64, chunk 2j+1]
        WvP_sb = consts.tile([128, NPAIR, 128], F16)
        nc.sync.dma_start(out=WvP_sb.rearrange("p j q -> p (j q)"), in_=WvP)
        gv_rep = consts.tile([128, NSLOT], F32)
        nc.sync.dma_start(out=gv_rep, in_=gv.to_broadcast((128, NSLOT)))
        ones16 = consts.tile([NSLOT + 1, 128], F16)
        nc.vector.memset(ones16, 1.0)

        def phase_A_dma(t):
            """prefetch x tile + xl + xh8 (dim-chunks 0-7)."""
            st = {}
            x_sb = xpool.tile([128, NCH, TOK], F16, tag="x_sb")
            nc.sync.dma_start(
                out=x_sb.rearrange("p c k -> p (c k)"), in_=xt[:, ts(t, FT)]
            )
            xq8 = qpool.tile([128, 2 * NCH, TOK], F8, tag="xq8")
            nc.sync.dma_start(
                out=xq8[:, 0:NCH, :].rearrange("p c k -> p (c k)"),
                in_=xlt[:, ts(t, FT)],
            )
            nc.sync.dma_start(
                out=xq8[:, NCH : NCH + NCH // 2, :].rearrange("p c k -> p (c k)"),
                in_=xht[:, ts(t, FT // 2)],
            )
            st["x_sb"] = x_sb
            st["xq8"] = xq8
            return st

        def act_cvt(st):
            """ACT converts dim-chunks 8-15 of xh -> fp8."""
            xq8, x_sb = st["xq8"], st["x_sb"]
            for h in range(2, 4):
                nc.scalar.copy(
                    xq8[:, NCH + 4 * h : NCH + 4 * h + 4, :],
                    x_sb[:, 4 * h : 4 * h + 4, :],
                )

        def S_corr_mms(st, S2_ps):
            """fp8 DoubleRow pass: (xl*2^10)@e4m3(Gh) + e4m3(xh)@(Gl*2^10)."""
            xq8 = st["xq8"]
            for i in range(NCH):
                nc.tensor.matmul(
                    S2_ps[0:MPAD, :],
                    GC_sb[:, 2 * i : 2 * i + 2, :],
                    xq8[:, 2 * i : 2 * i + 2, :],
                    start=(i == 0),
                    stop=(i == NCH - 1),
                    perf_mode=DR,
                    skip_group_check=True,
                )

        def S_merge(st, S_ps, S2_ps):
            """S_sb = main + corr * 2^-10, staged for the transposes."""
            S_c = work.tile([NSLOT + 1, TOK], F32, tag="S_c")
            nc.scalar.activation(
                S_c, S2_ps[0 : NSLOT + 1, :], func=ACTF.Copy, scale=1.0 / CSCALE
            )
            S_sb = work.tile([NSLOT + 1, TOK], F32, tag="S_sb")
            nc.vector.tensor_add(S_sb, S_ps[0 : NSLOT + 1, :], S_c)
            Stok = psT.tile([128, NQ, NSLOT + 1], F32, tag="T")
            for q in range(NQ):
                nc.tensor.transpose(
                    Stok[:, q, :],
                    S_sb[:, ts(q, 128)],
                    ident[0 : NSLOT + 1, 0 : NSLOT + 1],
                )
            st["Stok"] = Stok

        def phase_A_mm(t, st):
            """S matmuls (main fp16 + fp8 corr) -> merge -> transposes."""
            x_sb = st["x_sb"]
            S_ps = psA.tile([128, TOK], F32, tag="A")
            for c in range(NCH):
                nc.tensor.matmul(
                    S_ps[0 : NSLOT + 1, :],
                    GT_sb[:, c, :],
                    x_sb[:, c, :],
                    start=(c == 0),
                    stop=(c == NCH - 1),
                )
            S2_ps = psA2.tile([128, TOK], F32, tag="A2")
            S_corr_mms(st, S2_ps)
            S_merge(st, S_ps, S2_ps)

        def phase_B(t, st):
            """Batched softmax/gate stats; Ec' = -E*r*en, g in row 64."""
            Stok = st["Stok"]
            Etok = [
                small.tile([128, NSLOT], F32, tag=f"Etok{q}", name=f"Etok{q}")
                for q in range(NQ)
            ]
            Ec = small.tile([128, NQ, NSLOT + 1], F32, tag="Ec")
            scr = small.tile([128, NSLOT], F32, tag="scr")
            mb4 = small.tile([128, NQ], F32, tag="mb4")
            sums4 = small.tile([128, NQ], F32, tag="sums4")
            gvd4 = small.tile([128, NQ], F32, tag="gvd4")
            st4 = small.tile([128, 6, NQ], F32, tag="st4")
            mx4, r4, t4, gl4, en4, g4 = (st4[:, i, :] for i in range(6))
            cp4 = small.tile([128, NQ], F32, tag="cp4")
            nc.vector.tensor_reduce(mx4, Stok[:, :, 0:NSLOT], axis=AX_X.X, op=ALU.max)
            nc.vector.tensor_scalar_mul(mb4, mx4, -10.0)
            for q in range(NQ):
                nc.scalar.activation(
                    Etok[q],
                    Stok[:, q, 0:NSLOT],
                    func=ACTF.Exp,
                    bias=mb4[:, q : q + 1],
                    scale=10.0,
                    accum_out=sums4[:, q : q + 1],
                )
            for q in range(NQ):
                nc.vector.tensor_mul(scr, Etok[q], gv_rep)
                nc.vector.tensor_reduce(
                    gvd4[:, q : q + 1], scr, axis=AX_X.X, op=ALU.add
                )
            nc.vector.reciprocal(r4, sums4)
            nc.vector.tensor_mul(t4, gvd4, r4)
            nc.vector.tensor_add(gl4, t4, Stok[:, :, NSLOT])
            nc.scalar.activation(en4, gl4, func=ACTF.Exp, bias=-gate_b, scale=-1.0)
            nc.vector.tensor_scalar_add(g4, en4, 1.0)
            nc.vector.reciprocal(g4, g4)
            nc.vector.tensor_mul(cp4, r4, en4)
            nc.vector.tensor_scalar_mul(cp4, cp4, -1.0)
            for q in range(NQ):
                nc.vector.tensor_scalar_mul(
                    Ec[:, q, 0:NSLOT], Etok[q], cp4[:, q : q + 1]
                )
            for q in range(NQ):
                nc.vector.tensor_copy(Ec[:, q, NSLOT : NSLOT + 1], g4[:, q : q + 1])
            st["Ec"] = Ec

        def phase_C_pre(t, st):
            """Ec -> slot-major (+dup to partitions 64-127), g broadcast.

            Issued BEFORE phase_A_dma(t+3) so the tiny E2-dup DMA lands on
            the sync queue ahead of the 3 MiB bulk prefetch.
            """
            Ec = st["Ec"]
            ET = psE.tile([NSLOT + 1, NQ, 128], F32, tag="E")
            for q in range(NQ):
                nc.tensor.transpose(ET[:, q, :], Ec[:, q, :], ident)
            E_sb = work.tile([NSLOT + 1, NQ, 128], F16, tag="E_sb")
            nc.scalar.copy(E_sb, ET)
            # duplicate slot rows to partitions 64-127 for the row-tiled pair
            # (idle gpsimd SWDGE queue: never queues behind bulk transfers)
            E2 = work.tile([128, NQ, 128], F16, tag="E2")
            nc.gpsimd.dma_start(out=E2[64:128, :, :], in_=E_sb[0:NSLOT, :, :])
            # broadcast the gate row (partition 64) to all partitions via a
            # ones-matmul into the psA2 bank (free between the corr-merge
            # read and the next tile's DR writes)
            g_bc = psA2.tile([128, TOK], F32, tag="A2")
            nc.tensor.matmul(
                g_bc,
                ones16[NSLOT : NSLOT + 1, :],
                E_sb.rearrange("p a b -> p (a b)")[NSLOT : NSLOT + 1, :],
                start=True,
                stop=True,
                skip_group_check=True,
            )
            g_sb = work.tile([128, TOK], F32, tag="g_sb")
            nc.scalar.copy(g_sb, g_bc)
            st["E_sb"], st["E2"], st["g_sb"] = E_sb, E2, g_sb

        def phase_C(t, st, s_next=None):
            """Row-tiled pairs: PSUM = xh + Ec'@WvN ; out = g*PSUM ; one out
            DMA per tile.

            s_next=(t2, st2): interleave tile t2's S matmuls between this
            tile's I/R matmuls so PE stays busy while DVE paces the combine.
            """
            x_sb = st["x_sb"]
            E_flat = st["E_sb"].rearrange("p a b -> p (a b)")  # [65, 512]
            E2_flat = st["E2"].rearrange("p a b -> p (a b)")  # rows 64:128
            g_sb = st["g_sb"]
            if s_next is not None:
                t2, st2 = s_next
                S_ps2 = psA.tile([128, TOK], F32, tag="A")
                st["S_ps2"] = S_ps2
            o16 = opool.tile([128, NCH, TOK], F16, tag="o16")
            pend = []  # (cA, R_A, cB, R_B) with combine lagging one pair

            def drain_pair():
                cA, R_A, cB, R_B = pend.pop(0)
                nc.vector.tensor_mul(o16[:, cA, :], R_A, g_sb)
                nc.vector.tensor_mul(o16[:, cB, :], R_B, g_sb)
                if cB % 4 == 3:
                    # quarter-tile out-DMA right after its chunks complete:
                    # a whole-tile DMA head-blocked its queue ~20us waiting
                    # on the final DVE mul.
                    q4 = cB // 4
                    nc.scalar.dma_start(
                        out=outt[:, ts(t * 4 + q4, FT // 4)],
                        in_=o16[:, 4 * q4 : 4 * q4 + 4, :].rearrange(
                            "p c k -> p (c k)"
                        ),
                    )

            for j in range(NPAIR):
                cA, cB = 2 * j, 2 * j + 1
                R_A = psR.tile([128, TOK], F32, tag="R")
                R_B = psR.tile([128, TOK], F32, tag="R")
                nc.tensor.matmul(
                    R_A, ident16, x_sb[:, cA, :],
                    start=True, stop=False, skip_group_check=True,
                )
                nc.tensor.matmul(
                    R_B, ident16, x_sb[:, cB, :],
                    start=True, stop=False, skip_group_check=True,
                )
                nc.tensor.matmul(
                    R_A, WvP_sb[0:NSLOT, j, :], E_flat[0:NSLOT, :],
                    start=False, stop=True, skip_group_check=True,
                )
                nc.tensor.matmul(
                    R_B, WvP_sb[NSLOT:128, j, :], E2_flat[NSLOT:128, :],
                    start=False, stop=True, skip_group_check=True,
                )
                if s_next is not None:
                    for c in (cA, cB):
                        nc.tensor.matmul(
                            S_ps2[0 : NSLOT + 1, :],
                            GT_sb[:, c, :],
                            st2["x_sb"][:, c, :],
                            start=(c == 0),
                            stop=(c == NCH - 1),
                            skip_group_check=True,
                        )
                pend.append((cA, R_A, cB, R_B))
                if len(pend) > 1:
                    drain_pair()
            while pend:
                drain_pair()

        def phase_C_tail(t, st, s_next):
            """s_next's DR corr + merge + Stok transposes (PE-queue tail)."""
            if s_next is not None:
                t2, st2 = s_next
                S2_ps2 = psA2.tile([128, TOK], F32, tag="A2")
                S_corr_mms(st2, S2_ps2)
                S_merge(st2, st["S_ps2"], S2_ps2)

        # software pipeline, 4 tiles deep: x-DMA 4 tiles ahead; tile t+3's
        # S/DR matmuls interleave into tile t's combine; phase_B(t+2) —
        # whose Stok inputs were produced LAST iteration — is issued
        # before the PE-dense tail so the DVE overlaps it with the S/DR
        # block instead of colliding with the pair loop's combine muls.
        states = {}
        for u in range(4):
            states[u] = phase_A_dma(u)
        for u in range(4):
            act_cvt(states[u])
        phase_A_mm(0, states[0])
        phase_B(0, states[0])
        phase_A_mm(1, states[1])
        phase_B(1, states[1])
        phase_A_mm(2, states[2])
        phase_C_pre(0, states[0])
        for t in range(NT):
            if t + 4 < NT:
                states[t + 4] = phase_A_dma(t + 4)
            s_next = (t + 3, states[t + 3]) if t + 3 < NT else None
            phase_C(t, states[t], s_next=s_next)
            if t + 1 < NT:
                phase_C_pre(t + 1, states[t + 1])
            if t + 2 < NT:
                phase_B(t + 2, states[t + 2])
            phase_C_tail(t, states[t], s_next)
            if t + 4 < NT:
                act_cvt(states[t + 4])
            del states[t]

    nc.compile()
    return nc


def _fold_weights(memory, key_w, value_w, gate_w):
    """Fold module weights; returns un-packed (GT, GC, WvN, gv) as in v3."""
    mem = memory.astype(np.float64)
    Ws = (mem @ key_w.astype(np.float64)).astype(np.float32)  # [64, 2048]
    Wv = (mem @ value_w.astype(np.float64).T).astype(np.float32)  # [64, 2048]
    gx = np.asarray(gate_w[0, :DIM], dtype=np.float32)
    gvv = (Wv.astype(np.float64) @ gate_w[0, DIM:].astype(np.float64)).astype(
        np.float32
    )
    G = np.concatenate([Ws, gx[None, :]], axis=0)  # [65, 2048]; gate row last
    WvN = np.ascontiguousarray(-Wv).astype(np.float16)  # [64, 2048]
    F8NP = mybir.dt.np(F8)
    Gh = G.astype(np.float16)  # [65, 2048]
    Gh8 = Gh.astype(F8NP)
    Gl10 = ((G - Gh.astype(np.float32)) * CSCALE).astype(F8NP)
    GT = np.ascontiguousarray(Gh.T)  # [2048, 65] fp16
    GC = np.zeros((2 * DIM, MPAD), dtype=F8NP)
    GC[:DIM, : NSLOT + 1] = Gh8.T
    GC[DIM:, : NSLOT + 1] = Gl10.T
    return GT, GC, WvN, gvv.reshape(1, NSLOT)


def _pack_weights(GT, GC, WvN):
    """Device layouts: [128 partitions, contiguous free]."""
    # GT [2048, 65] -> [128, NCH*65]; row d = c*128+p
    GTt = np.ascontiguousarray(
        GT.reshape(NCH, 128, NSLOT + 1).transpose(1, 0, 2).reshape(128, -1)
    )
    # GC [4096, 80] -> [128, 32*80]
    GCt = np.ascontiguousarray(
        GC.reshape(2 * NCH, 128, MPAD).transpose(1, 0, 2).reshape(128, -1)
    )
    # WvN [64, 2048] -> WvP [128, NPAIR*128]
    Wv3 = WvN.reshape(NSLOT, NCH, 128)
    WvP = np.concatenate([Wv3[:, 0::2, :], Wv3[:, 1::2, :]], axis=0)
    WvP = np.ascontiguousarray(WvP.reshape(128, -1))
    return GTt, GCt, WvP


def _pack_x(xb):
    """x [L, DIM] f32 -> (xt f16, xlt f8, xht f8 [dim-chunks 0-7])."""
    F8NP = mybir.dt.np(F8)
    xT = np.ascontiguousarray(xb.T)  # [2048, 4096]
    xh = xT.astype(np.float16)
    xl8 = ((xT - xh.astype(np.float32)) * CSCALE).astype(F8NP)
    # [d= c*128+p, tok= t*512+k] -> [p, t, c, k]
    xh4 = xh.reshape(NCH, 128, NT, TOK)
    xtp = np.ascontiguousarray(xh4.transpose(1, 2, 0, 3).reshape(128, -1))
    xlp = np.ascontiguousarray(
        xl8.reshape(NCH, 128, NT, TOK).transpose(1, 2, 0, 3).reshape(128, -1)
    )
    xhp = np.ascontiguousarray(
        xh4[: NCH // 2].astype(F8NP).transpose(1, 2, 0, 3).reshape(128, -1)
    )
    return xtp, xlp, xhp


def _unpack_out(o):
    """outt [128, NT*FT] f16 -> out [L, DIM] f32."""
    # [p, t, c, k] -> [d= c*128+p, tok= t*512+k]
    oT = o.reshape(128, NT, NCH, TOK).transpose(2, 0, 1, 3).reshape(DIM, L)
    return oT.T.astype(np.float32)


def kernel(x, memory, key_w, value_w, gate_w, gate_b, _trace=False, _tmpdir=None):
    x = np.asarray(x, dtype=np.float32)
    GT, GC, WvN, gvv = _fold_weights(
        np.asarray(memory, np.float32),
        np.asarray(key_w, np.float32),
        np.asarray(value_w, np.float32),
        np.asarray(gate_w, np.float32),
    )
    GTt, GCt, WvP = _pack_weights(GT, GC, WvN)
    nc = _build(float(np.asarray(gate_b).reshape(-1)[0]))
    in_maps = []
    for b in range(B):
        xtp, xlp, xhp = _pack_x(x[b])
        in_maps.append(
            {"xt": xtp, "xlt": xlp, "xht": xhp,
             "GTt": GTt, "GCt": GCt, "WvP": WvP, "gv": gvv}
        )
    res = run_bass_kernel_spmd(
        nc, in_maps, list(range(B)), trace=_trace, tmpdir=_tmpdir
    )
    out = np.stack(
        [_unpack_out(res.results[b]["outt"]) for b in range(B)], axis=0
    )
    if _trace:
        return out, res
    return out


def sim_core0(inputs, expected):
    """CoreSim check of core 0 against expected[0]; returns maxabs err."""
    from concourse.bass_interp import CoreSim

    GT, GC, WvN, gvv = _fold_weights(
        inputs["memory"], inputs["key_w"], inputs["value_w"], inputs["gate_w"]
    )
    GTt, GCt, WvP = _pack_weights(GT, GC, WvN)
    nc = _build(float(inputs["gate_b"][0]))
    sim = CoreSim(nc)
    xtp, xlp, xhp = _pack_x(inputs["x"][0])
    sim.tensor("xt")[:] = xtp
    sim.tensor("xlt")[:] = xlp
    sim.tensor("xht")[:] = xhp
    sim.tensor("GTt")[:] = GTt
    sim.tensor("GCt")[:] = GCt
    sim.tensor("WvP")[:] = WvP
    sim.tensor("gv")[:] = gvv
    sim.simulate()
    got = _unpack_out(np.asarray(sim.tensor("outt")))
    return np.abs(got - expected[0]).max()


# revision 33
# speedup vs baseline: 1.0998x; 1.0458x over previous
"""MemoryBank kernel v4 for 8x TRN2 NeuronCores (SPMD, batch-parallel).

Same folded algebra as v3 (split-precision scores, exp-based gate fold):

    x  = xh (fp16, DMA'd) + xl (e4m3 * 2^-10, DMA'd)
    S  = xh@Gh  (fp16 full-rate)
       + 2^-10 * [ (xl*2^10)@e4m3(Gh) + e4m3(xh)@(Gl*2^10) ]   (fp8 DoubleRow)
    en = exp(-gate_logit); g = 1/(1+en); Ec' = -E*r*en
    PSUM = xh + Ec'@WvN   (identity-pass + retrieval on PE)
    out  = g * PSUM       (single DVE op per element, fp16 out)

v4 changes vs v3 (227 us):
  - xh8 derived ON-CHIP (8 chunks gpsimd + 8 chunks ACT converting copies)
    instead of DMA'd: HBM traffic 48.8 -> 40.3 MiB.
  - pre-tiled DRAM layouts: every stream is [128 part, contiguous] per
    tile (16 KiB runs vs 1 KiB), collapsing descriptor count ~16x and
    un-blocking the sync queue (110 us of DMA_DIRECT2D issuance in v3).
  - retrieval row-tiled in pairs: Wv chunk 2j in PE rows 0-63, chunk
    2j+1 in rows 64-127 (Ec duplicated to partitions 64-127 by a tiny
    SBUF->SBUF DMA); 16 -> 8 retrieval pass-slots per tile.
  - S-merge fused into one DVE scalar_tensor_tensor; gvd via
    tensor_tensor_reduce.

DMA floor: (16+8+16.3) MiB / 358 GB/s ~= 118 us/core.
"""

from contextlib import ExitStack

import numpy as np

import concourse.bass as bass
import concourse.tile as tile
from concourse import bacc
from concourse import mybir
from concourse.bass import ts
from concourse.bass_utils import run_bass_kernel_spmd
from concourse.masks import make_identity

F32 = mybir.dt.float32
F16 = mybir.dt.float16
F8 = mybir.dt.float8e4
AX_X = mybir.AxisListType
ALU = mybir.AluOpType
ACTF = mybir.ActivationFunctionType
DR = mybir.MatmulPerfMode.DoubleRow

B = 8
L = 4096
DIM = 2048
NSLOT = 64
NCH = DIM // 128  # 16 dim chunks
TOK = 512  # tokens per tile
NT = L // TOK  # 8 tiles per core
NQ = TOK // 128  # 4 token quarters per tile
NPAIR = NCH // 2  # 8 retrieval chunk pairs
CSCALE = 1024.0  # 2^10 scale of the fp8 correction pass
MPAD = 80  # fp8 DoubleRow weight APs need step%16==0, so pad 65 -> 80 cols
FT = NCH * TOK  # flat free size of one tile per partition


def _build(gate_b: float) -> bass.Bass:
    nc = bacc.Bacc("TRN2", target_bir_lowering=False, debug=False)

    xt = nc.dram_tensor("xt", [128, NT * FT], F16, kind="ExternalInput").ap()
    xlt = nc.dram_tensor("xlt", [128, NT * FT], F8, kind="ExternalInput").ap()
    # xh8 for dim-chunks 0-7 only; chunks 8-15 are converted on ACT
    xht = nc.dram_tensor("xht", [128, NT * FT // 2], F8, kind="ExternalInput").ap()
    GTt = nc.dram_tensor(
        "GTt", [128, NCH * (NSLOT + 1)], F16, kind="ExternalInput"
    ).ap()
    GCt = nc.dram_tensor("GCt", [128, 2 * NCH * MPAD], F8, kind="ExternalInput").ap()
    WvP = nc.dram_tensor("WvP", [128, NPAIR * 128], F16, kind="ExternalInput").ap()
    gv = nc.dram_tensor("gv", [1, NSLOT], F32, kind="ExternalInput").ap()
    outt = nc.dram_tensor("outt", [128, NT * FT], F16, kind="ExternalOutput").ap()

    with tile.TileContext(nc) as tc, ExitStack() as ctx:
        consts = ctx.enter_context(tc.tile_pool(name="consts", bufs=1))
        xpool = ctx.enter_context(tc.tile_pool(name="xpool", bufs=5))
        qpool = ctx.enter_context(tc.tile_pool(name="qpool", bufs=3))
        opool = ctx.enter_context(tc.tile_pool(name="opool", bufs=2))
        work = ctx.enter_context(tc.tile_pool(name="work", bufs=3))
        small = ctx.enter_context(tc.tile_pool(name="small", bufs=3))
        psA = ctx.enter_context(tc.tile_pool(name="psA", bufs=1, space="PSUM"))
        psA2 = ctx.enter_context(tc.tile_pool(name="psA2", bufs=1, space="PSUM"))
        psT = ctx.enter_context(tc.tile_pool(name="psT", bufs=1, space="PSUM"))
        psE = ctx.enter_context(tc.tile_pool(name="psE", bufs=1, space="PSUM"))
        psR = ctx.enter_context(tc.tile_pool(name="psR", bufs=4, space="PSUM"))

        ident = consts.tile([128, 128], F32)
        make_identity(nc, ident)
        ident16 = consts.tile([128, 128], F16)
        nc.scalar.copy(ident16, ident)
        GT_sb = consts.tile([128, NCH, NSLOT + 1], F16)
        nc.sync.dma_start(
            out=GT_sb.rearrange("p c m -> p (c m)"), in_=GTt
        )
        GC_sb = consts.tile([128, 2 * NCH, MPAD], F8)
        nc.sync.dma_start(out=GC_sb.rearrange("p s m -> p (s m)"), in_=GCt)
        # WvP_sb[p<64, j, :]  = -Wv[slot p,  chunk 2j]
        # WvP_sb[p>=64, j, :] = -Wv[slot p-64, chunk 2j+1]
        WvP_sb = consts.tile([128, NPAIR, 128], F16)
        nc.sync.dma_start(out=WvP_sb.rearrange("p j q -> p (j q)"), in_=WvP)
        gv_rep = consts.tile([128, NSLOT], F32)
        nc.sync.dma_start(out=gv_rep, in_=gv.to_broadcast((128, NSLOT)))
        ones16 = consts.tile([NSLOT + 1, 128], F16)
        nc.vector.memset(ones16, 1.0)

        def phase_A_dma(t):
            """prefetch x tile + xl + xh8 (dim-chunks 0-7)."""
            st = {}
            x_sb = xpool.tile([128, NCH, TOK], F16, tag="x_sb")
            nc.sync.dma_start(
                out=x_sb.rearrange("p c k -> p (c k)"), in_=xt[:, ts(t, FT)]
            )
            xq8 = qpool.tile([128, 2 * NCH, TOK], F8, tag="xq8")
            nc.sync.dma_start(
                out=xq8[:, 0:NCH, :].rearrange("p c k -> p (c k)"),
                in_=xlt[:, ts(t, FT)],
            )
            nc.sync.dma_start(
                out=xq8[:, NCH : NCH + NCH // 2, :].rearrange("p c k -> p (c k)"),
                in_=xht[:, ts(t, FT // 2)],
            )
            st["x_sb"] = x_sb
            st["xq8"] = xq8
            return st

        def act_cvt(st):
            """ACT converts dim-chunks 8-15 of xh -> fp8."""
            xq8, x_sb = st["xq8"], st["x_sb"]
            for h in range(2, 4):
                nc.scalar.copy(
                    xq8[:, NCH + 4 * h : NCH + 4 * h + 4, :],
                    x_sb[:, 4 * h : 4 * h + 4, :],
                )

        def S_corr_mms(st, S2_ps):
            """fp8 DoubleRow pass: (xl*2^10)@e4m3(Gh) + e4m3(xh)@(Gl*2^10)."""
            xq8 = st["xq8"]
            for i in range(NCH):
                nc.tensor.matmul(
                    S2_ps[0:MPAD, :],
                    GC_sb[:, 2 * i : 2 * i + 2, :],
                    xq8[:, 2 * i : 2 * i + 2, :],
                    start=(i == 0),
                    stop=(i == NCH - 1),
                    perf_mode=DR,
                    skip_group_check=True,
                )

        def S_merge(st, S_ps, S2_ps):
            """S_sb = main + corr * 2^-10, staged for the transposes."""
            S_c = work.tile([NSLOT + 1, TOK], F32, tag="S_c")
            nc.scalar.activation(
                S_c, S2_ps[0 : NSLOT + 1, :], func=ACTF.Copy, scale=1.0 / CSCALE
            )
            S_sb = work.tile([NSLOT + 1, TOK], F32, tag="S_sb")
            nc.vector.tensor_add(S_sb, S_ps[0 : NSLOT + 1, :], S_c)
            Stok = psT.tile([128, NQ, NSLOT + 1], F32, tag="T")
            for q in range(NQ):
                nc.tensor.transpose(
                    Stok[:, q, :],
                    S_sb[:, ts(q, 128)],
                    ident[0 : NSLOT + 1, 0 : NSLOT + 1],
                )
            st["Stok"] = Stok

        def phase_A_mm(t, st):
            """S matmuls (main fp16 + fp8 corr) -> merge -> transposes."""
            x_sb = st["x_sb"]
            S_ps = psA.tile([128, TOK], F32, tag="A")
            for c in range(NCH):
                nc.tensor.matmul(
                    S_ps[0 : NSLOT + 1, :],
                    GT_sb[:, c, :],
                    x_sb[:, c, :],
                    start=(c == 0),
                    stop=(c == NCH - 1),
                )
            S2_ps = psA2.tile([128, TOK], F32, tag="A2")
            S_corr_mms(st, S2_ps)
            S_merge(st, S_ps, S2_ps)

        def phase_B(t, st):
            """Batched softmax/gate stats; Ec' = -E*r*en, g in row 64."""
            Stok = st["Stok"]
            Etok = [
                small.tile([128, NSLOT], F32, tag=f"Etok{q}", name=f"Etok{q}")
                for q in range(NQ)
            ]
            Ec = small.tile([128, NQ, NSLOT + 1], F32, tag="Ec")
            scr = small.tile([128, NSLOT], F32, tag="scr")
            mb4 = small.tile([128, NQ], F32, tag="mb4")
            sums4 = small.tile([128, NQ], F32, tag="sums4")
            gvd4 = small.tile([128, NQ], F32, tag="gvd4")
            st4 = small.tile([128, 6, NQ], F32, tag="st4")
            mx4, r4, t4, gl4, en4, g4 = (st4[:, i, :] for i in range(6))
            cp4 = small.tile([128, NQ], F32, tag="cp4")
            nc.vector.tensor_reduce(mx4, Stok[:, :, 0:NSLOT], axis=AX_X.X, op=ALU.max)
            nc.vector.tensor_scalar_mul(mb4, mx4, -10.0)
            for q in range(NQ):
                nc.scalar.activation(
                    Etok[q],
                    Stok[:, q, 0:NSLOT],
                    func=ACTF.Exp,
                    bias=mb4[:, q : q + 1],
                    scale=10.0,
                    accum_out=sums4[:, q : q + 1],
                )
            for q in range(NQ):
                nc.vector.tensor_mul(scr, Etok[q], gv_rep)
                nc.vector.tensor_reduce(
                    gvd4[:, q : q + 1], scr, axis=AX_X.X, op=ALU.add
                )
            nc.vector.reciprocal(r4, sums4)
            nc.vector.tensor_mul(t4, gvd4, r4)
            nc.vector.tensor_add(gl4, t4, Stok[:, :, NSLOT])
            nc.scalar.activation(en4, gl4, func=ACTF.Exp, bias=-gate_b, scale=-1.0)
            nc.vector.tensor_scalar_add(g4, en4, 1.0)
            nc.vector.reciprocal(g4, g4)
            nc.vector.tensor_mul(cp4, r4, en4)
            nc.vector.tensor_scalar_mul(cp4, cp4, -1.0)
            for q in range(NQ):
                nc.vector.tensor_scalar_mul(
                    Ec[:, q, 0:NSLOT], Etok[q], cp4[:, q : q + 1]
                )
            for q in range(NQ):
                nc.vector.tensor_copy(Ec[:, q, NSLOT : NSLOT + 1], g4[:, q : q + 1])
            st["Ec"] = Ec

        def phase_C_pre(t, st):
            """Ec -> slot-major (+dup to partitions 64-127), g broadcast.

            Issued BEFORE phase_A_dma(t+3) so the tiny E2-dup DMA lands on
            the sync queue ahead of the 3 MiB bulk prefetch.
            """
            Ec = st["Ec"]
            ET = psE.tile([NSLOT + 1, NQ, 128], F32, tag="E")
            for q in range(NQ):
                nc.tensor.transpose(ET[:, q, :], Ec[:, q, :], ident)
            E_sb = work.tile([NSLOT + 1, NQ, 128], F16, tag="E_sb")
            nc.scalar.copy(E_sb, ET)
            # duplicate slot rows to partitions 64-127 for the row-tiled pair
            # (idle gpsimd SWDGE queue: never queues behind bulk transfers)
            E2 = work.tile([128, NQ, 128], F16, tag="E2")
            nc.gpsimd.dma_start(out=E2[64:128, :, :], in_=E_sb[0:NSLOT, :, :])
            # broadcast the gate row (partition 64) to all partitions via a
            # ones-matmul into the psA2 bank (free between the corr-merge
            # read and the next tile's DR writes)
            g_bc = psA2.tile([128, TOK], F32, tag="A2")
            nc.tensor.matmul(
                g_bc,
                ones16[NSLOT : NSLOT + 1, :],
                E_sb.rearrange("p a b -> p (a b)")[NSLOT : NSLOT + 1, :],
                start=True,
                stop=True,
                skip_group_check=True,
            )
            g_sb = work.tile([128, TOK], F32, tag="g_sb")
            nc.scalar.copy(g_sb, g_bc)
            st["E_sb"], st["E2"], st["g_sb"] = E_sb, E2, g_sb

        def phase_C(t, st, s_next=None):
            """Row-tiled pairs: PSUM = xh + Ec'@WvN ; out = g*PSUM ; one out
            DMA per tile.

            s_next=(t2, st2): interleave tile t2's S matmuls between this
            tile's I/R matmuls so PE stays busy while DVE paces the combine.
            """
            x_sb = st["x_sb"]
            E_flat = st["E_sb"].rearrange("p a b -> p (a b)")  # [65, 512]
            E2_flat = st["E2"].rearrange("p a b -> p (a b)")  # rows 64:128
            g_sb = st["g_sb"]
            if s_next is not None:
                t2, st2 = s_next
                S_ps2 = psA.tile([128, TOK], F32, tag="A")
                st["S_ps2"] = S_ps2
            o16 = opool.tile([128, NCH, TOK], F16, tag="o16")
            pend = []  # (cA, R_A, cB, R_B) with combine lagging one pair

            def drain_pair():
                cA, R_A, cB, R_B = pend.pop(0)
                nc.vector.tensor_mul(o16[:, cA, :], R_A, g_sb)
                nc.vector.tensor_mul(o16[:, cB, :], R_B, g_sb)
                if cB % 4 == 3:
                    # quarter-tile out-DMA right after its chunks complete,
                    # on the gpsimd SWDGE queue: on scalar/sync its wait for
                    # the DVE muls head-blocked everything behind it.
                    q4 = cB // 4
                    nc.gpsimd.dma_start(
                        out=outt[:, ts(t * 4 + q4, FT // 4)],
                        in_=o16[:, 4 * q4 : 4 * q4 + 4, :].rearrange(
                            "p c k -> p (c k)"
                        ),
                    )

            for j in range(NPAIR):
                cA, cB = 2 * j, 2 * j + 1
                R_A = psR.tile([128, TOK], F32, tag="R")
                R_B = psR.tile([128, TOK], F32, tag="R")
                nc.tensor.matmul(
                    R_A, ident16, x_sb[:, cA, :],
                    start=True, stop=False, skip_group_check=True,
                )
                nc.tensor.matmul(
                    R_B, ident16, x_sb[:, cB, :],
                    start=True, stop=False, skip_group_check=True,
                )
                nc.tensor.matmul(
                    R_A, WvP_sb[0:NSLOT, j, :], E_flat[0:NSLOT, :],
                    start=False, stop=True, skip_group_check=True,
                )
                nc.tensor.matmul(
                    R_B, WvP_sb[NSLOT:128, j, :], E2_flat[NSLOT:128, :],
                    start=False, stop=True, skip_group_check=True,
                )
                if s_next is not None:
                    for c in (cA, cB):
                        nc.tensor.matmul(
                            S_ps2[0 : NSLOT + 1, :],
                            GT_sb[:, c, :],
                            st2["x_sb"][:, c, :],
                            start=(c == 0),
                            stop=(c == NCH - 1),
                            skip_group_check=True,
                        )
                pend.append((cA, R_A, cB, R_B))
                if len(pend) > 1:
                    drain_pair()
            while pend:
                drain_pair()

        def phase_C_tail(t, st, s_next):
            """s_next's DR corr + merge + Stok transposes (PE-queue tail)."""
            if s_next is not None:
                t2, st2 = s_next
                S2_ps2 = psA2.tile([128, TOK], F32, tag="A2")
                S_corr_mms(st2, S2_ps2)
                S_merge(st2, st["S_ps2"], S2_ps2)

        # software pipeline, 4 tiles deep: x-DMA 4 tiles ahead; tile t+3's
        # S/DR matmuls interleave into tile t's combine; phase_B(t+2) —
        # whose Stok inputs were produced LAST iteration — is issued
        # before the PE-dense tail so the DVE overlaps it with the S/DR
        # block instead of colliding with the pair loop's combine muls.
        states = {}
        for u in range(4):
            states[u] = phase_A_dma(u)
        for u in range(4):
            act_cvt(states[u])
        phase_A_mm(0, states[0])
        phase_B(0, states[0])
        phase_A_mm(1, states[1])
        phase_B(1, states[1])
        phase_A_mm(2, states[2])
        phase_C_pre(0, states[0])
        for t in range(NT):
            if t + 4 < NT:
                states[t + 4] = phase_A_dma(t + 4)
            if t + 1 < NT:
                phase_C_pre(t + 1, states[t + 1])
            s_next = (t + 3, states[t + 3]) if t + 3 < NT else None
            phase_C(t, states[t], s_next=s_next)
            if t + 2 < NT:
                phase_B(t + 2, states[t + 2])
            phase_C_tail(t, states[t], s_next)
            if t + 4 < NT:
                act_cvt(states[t + 4])
            del states[t]

    nc.compile()
    return nc


def _fold_weights(memory, key_w, value_w, gate_w):
    """Fold module weights; returns un-packed (GT, GC, WvN, gv) as in v3."""
    mem = memory.astype(np.float64)
    Ws = (mem @ key_w.astype(np.float64)).astype(np.float32)  # [64, 2048]
    Wv = (mem @ value_w.astype(np.float64).T).astype(np.float32)  # [64, 2048]
    gx = np.asarray(gate_w[0, :DIM], dtype=np.float32)
    gvv = (Wv.astype(np.float64) @ gate_w[0, DIM:].astype(np.float64)).astype(
        np.float32
    )
    G = np.concatenate([Ws, gx[None, :]], axis=0)  # [65, 2048]; gate row last
    WvN = np.ascontiguousarray(-Wv).astype(np.float16)  # [64, 2048]
    F8NP = mybir.dt.np(F8)
    Gh = G.astype(np.float16)  # [65, 2048]
    Gh8 = Gh.astype(F8NP)
    Gl10 = ((G - Gh.astype(np.float32)) * CSCALE).astype(F8NP)
    GT = np.ascontiguousarray(Gh.T)  # [2048, 65] fp16
    GC = np.zeros((2 * DIM, MPAD), dtype=F8NP)
    GC[:DIM, : NSLOT + 1] = Gh8.T
    GC[DIM:, : NSLOT + 1] = Gl10.T
    return GT, GC, WvN, gvv.reshape(1, NSLOT)


def _pack_weights(GT, GC, WvN):
    """Device layouts: [128 partitions, contiguous free]."""
    # GT [2048, 65] -> [128, NCH*65]; row d = c*128+p
    GTt = np.ascontiguousarray(
        GT.reshape(NCH, 128, NSLOT + 1).transpose(1, 0, 2).reshape(128, -1)
    )
    # GC [4096, 80] -> [128, 32*80]
    GCt = np.ascontiguousarray(
        GC.reshape(2 * NCH, 128, MPAD).transpose(1, 0, 2).reshape(128, -1)
    )
    # WvN [64, 2048] -> WvP [128, NPAIR*128]
    Wv3 = WvN.reshape(NSLOT, NCH, 128)
    WvP = np.concatenate([Wv3[:, 0::2, :], Wv3[:, 1::2, :]], axis=0)
    WvP = np.ascontiguousarray(WvP.reshape(128, -1))
    return GTt, GCt, WvP


def _pack_x(xb):
    """x [L, DIM] f32 -> (xt f16, xlt f8, xht f8 [dim-chunks 0-7])."""
    F8NP = mybir.dt.np(F8)
    xT = np.ascontiguousarray(xb.T)  # [2048, 4096]
    xh = xT.astype(np.float16)
    xl8 = ((xT - xh.astype(np.float32)) * CSCALE).astype(F8NP)
    # [d= c*128+p, tok= t*512+k] -> [p, t, c, k]
    xh4 = xh.reshape(NCH, 128, NT, TOK)
    xtp = np.ascontiguousarray(xh4.transpose(1, 2, 0, 3).reshape(128, -1))
    xlp = np.ascontiguousarray(
        xl8.reshape(NCH, 128, NT, TOK).transpose(1, 2, 0, 3).reshape(128, -1)
    )
    xhp = np.ascontiguousarray(
        xh4[: NCH // 2].astype(F8NP).transpose(1, 2, 0, 3).reshape(128, -1)
    )
    return xtp, xlp, xhp


def _unpack_out(o):
    """outt [128, NT*FT] f16 -> out [L, DIM] f32."""
    # [p, t, c, k] -> [d= c*128+p, tok= t*512+k]
    oT = o.reshape(128, NT, NCH, TOK).transpose(2, 0, 1, 3).reshape(DIM, L)
    return oT.T.astype(np.float32)


def kernel(x, memory, key_w, value_w, gate_w, gate_b, _trace=False, _tmpdir=None):
    x = np.asarray(x, dtype=np.float32)
    GT, GC, WvN, gvv = _fold_weights(
        np.asarray(memory, np.float32),
        np.asarray(key_w, np.float32),
        np.asarray(value_w, np.float32),
        np.asarray(gate_w, np.float32),
    )
    GTt, GCt, WvP = _pack_weights(GT, GC, WvN)
    nc = _build(float(np.asarray(gate_b).reshape(-1)[0]))
    in_maps = []
    for b in range(B):
        xtp, xlp, xhp = _pack_x(x[b])
        in_maps.append(
            {"xt": xtp, "xlt": xlp, "xht": xhp,
             "GTt": GTt, "GCt": GCt, "WvP": WvP, "gv": gvv}
        )
    res = run_bass_kernel_spmd(
        nc, in_maps, list(range(B)), trace=_trace, tmpdir=_tmpdir
    )
    out = np.stack(
        [_unpack_out(res.results[b]["outt"]) for b in range(B)], axis=0
    )
    if _trace:
        return out, res
    return out


def sim_core0(inputs, expected):
    """CoreSim check of core 0 against expected[0]; returns maxabs err."""
    from concourse.bass_interp import CoreSim

    GT, GC, WvN, gvv = _fold_weights(
        inputs["memory"], inputs["key_w"], inputs["value_w"], inputs["gate_w"]
    )
    GTt, GCt, WvP = _pack_weights(GT, GC, WvN)
    nc = _build(float(inputs["gate_b"][0]))
    sim = CoreSim(nc)
    xtp, xlp, xhp = _pack_x(inputs["x"][0])
    sim.tensor("xt")[:] = xtp
    sim.tensor("xlt")[:] = xlp
    sim.tensor("xht")[:] = xhp
    sim.tensor("GTt")[:] = GTt
    sim.tensor("GCt")[:] = GCt
    sim.tensor("WvP")[:] = WvP
    sim.tensor("gv")[:] = gvv
    sim.simulate()
    got = _unpack_out(np.asarray(sim.tensor("outt")))
    return np.abs(got - expected[0]).max()


# revision 38
# speedup vs baseline: 1.1109x; 1.0101x over previous
"""MemoryBank kernel v4 for 8x TRN2 NeuronCores (SPMD, batch-parallel).

Same folded algebra as v3 (split-precision scores, exp-based gate fold):

    x  = xh (fp16, DMA'd) + xl (e4m3 * 2^-10, DMA'd)
    S  = xh@Gh  (fp16 full-rate)
       + 2^-10 * [ (xl*2^10)@e4m3(Gh) + e4m3(xh)@(Gl*2^10) ]   (fp8 DoubleRow)
    en = exp(-gate_logit); g = 1/(1+en); Ec' = -E*r*en
    PSUM = xh + Ec'@WvN   (identity-pass + retrieval on PE)
    out  = g * PSUM       (single DVE op per element, fp16 out)

v4 changes vs v3 (227 us):
  - xh8 derived ON-CHIP (8 chunks gpsimd + 8 chunks ACT converting copies)
    instead of DMA'd: HBM traffic 48.8 -> 40.3 MiB.
  - pre-tiled DRAM layouts: every stream is [128 part, contiguous] per
    tile (16 KiB runs vs 1 KiB), collapsing descriptor count ~16x and
    un-blocking the sync queue (110 us of DMA_DIRECT2D issuance in v3).
  - retrieval row-tiled in pairs: Wv chunk 2j in PE rows 0-63, chunk
    2j+1 in rows 64-127 (Ec duplicated to partitions 64-127 by a tiny
    SBUF->SBUF DMA); 16 -> 8 retrieval pass-slots per tile.
  - S-merge fused into one DVE scalar_tensor_tensor; gvd via
    tensor_tensor_reduce.

DMA floor: (16+8+16.3) MiB / 358 GB/s ~= 118 us/core.
"""

from contextlib import ExitStack

import numpy as np

import concourse.bass as bass
import concourse.tile as tile
from concourse import bacc
from concourse import mybir
from concourse.bass import ts
from concourse.bass_utils import run_bass_kernel_spmd
from concourse.masks import make_identity

F32 = mybir.dt.float32
F16 = mybir.dt.float16
F8 = mybir.dt.float8e4
AX_X = mybir.AxisListType
ALU = mybir.AluOpType
ACTF = mybir.ActivationFunctionType
DR = mybir.MatmulPerfMode.DoubleRow

B = 8
L = 4096
DIM = 2048
NSLOT = 64
NCH = DIM // 128  # 16 dim chunks
TOK = 512  # tokens per tile
NT = L // TOK  # 8 tiles per core
NQ = TOK // 128  # 4 token quarters per tile
NPAIR = NCH // 2  # 8 retrieval chunk pairs
CSCALE = 1024.0  # 2^10 scale of the fp8 correction pass
MPAD = 80  # fp8 DoubleRow weight APs need step%16==0, so pad 65 -> 80 cols
FT = NCH * TOK  # flat free size of one tile per partition


def _build(gate_b: float) -> bass.Bass:
    nc = bacc.Bacc("TRN2", target_bir_lowering=False, debug=False)

    xt = nc.dram_tensor("xt", [128, NT * FT], F16, kind="ExternalInput").ap()
    xlt = nc.dram_tensor("xlt", [128, NT * FT], F8, kind="ExternalInput").ap()
    # xh8 for dim-chunks 0-7 only; chunks 8-15 are converted on ACT
    xht = nc.dram_tensor("xht", [128, NT * FT // 2], F8, kind="ExternalInput").ap()
    GTt = nc.dram_tensor(
        "GTt", [128, NCH * (NSLOT + 1)], F16, kind="ExternalInput"
    ).ap()
    GCt = nc.dram_tensor("GCt", [128, 2 * NCH * MPAD], F8, kind="ExternalInput").ap()
    WvP = nc.dram_tensor("WvP", [128, NPAIR * 128], F16, kind="ExternalInput").ap()
    gv = nc.dram_tensor("gv", [1, NSLOT], F32, kind="ExternalInput").ap()
    outt = nc.dram_tensor("outt", [128, NT * FT], F16, kind="ExternalOutput").ap()

    with tile.TileContext(nc) as tc, ExitStack() as ctx:
        consts = ctx.enter_context(tc.tile_pool(name="consts", bufs=1))
        xpool = ctx.enter_context(tc.tile_pool(name="xpool", bufs=5))
        qpool = ctx.enter_context(tc.tile_pool(name="qpool", bufs=3))
        opool = ctx.enter_context(tc.tile_pool(name="opool", bufs=2))
        work = ctx.enter_context(tc.tile_pool(name="work", bufs=3))
        small = ctx.enter_context(tc.tile_pool(name="small", bufs=3))
        psA = ctx.enter_context(tc.tile_pool(name="psA", bufs=1, space="PSUM"))
        psA2 = ctx.enter_context(tc.tile_pool(name="psA2", bufs=1, space="PSUM"))
        psT = ctx.enter_context(tc.tile_pool(name="psT", bufs=1, space="PSUM"))
        psE = ctx.enter_context(tc.tile_pool(name="psE", bufs=1, space="PSUM"))
        psR = ctx.enter_context(tc.tile_pool(name="psR", bufs=4, space="PSUM"))

        ident = consts.tile([128, 128], F32)
        make_identity(nc, ident)
        ident16 = consts.tile([128, 128], F16)
        nc.scalar.copy(ident16, ident)
        GT_sb = consts.tile([128, NCH, NSLOT + 1], F16)
        nc.sync.dma_start(
            out=GT_sb.rearrange("p c m -> p (c m)"), in_=GTt
        )
        GC_sb = consts.tile([128, 2 * NCH, MPAD], F8)
        nc.sync.dma_start(out=GC_sb.rearrange("p s m -> p (s m)"), in_=GCt)
        # WvP_sb[p<64, j, :]  = -Wv[slot p,  chunk 2j]
        # WvP_sb[p>=64, j, :] = -Wv[slot p-64, chunk 2j+1]
        WvP_sb = consts.tile([128, NPAIR, 128], F16)
        nc.sync.dma_start(out=WvP_sb.rearrange("p j q -> p (j q)"), in_=WvP)
        gv_rep = consts.tile([128, NSLOT], F32)
        nc.sync.dma_start(out=gv_rep, in_=gv.to_broadcast((128, NSLOT)))
        ones16 = consts.tile([NSLOT + 1, 128], F16)
        nc.vector.memset(ones16, 1.0)

        def phase_A_dma(t):
            """prefetch x tile + xl + xh8 (dim-chunks 0-7)."""
            st = {}
            x_sb = xpool.tile([128, NCH, TOK], F16, tag="x_sb")
            nc.sync.dma_start(
                out=x_sb.rearrange("p c k -> p (c k)"), in_=xt[:, ts(t, FT)]
            )
            xq8 = qpool.tile([128, 2 * NCH, TOK], F8, tag="xq8")
            nc.sync.dma_start(
                out=xq8[:, 0:NCH, :].rearrange("p c k -> p (c k)"),
                in_=xlt[:, ts(t, FT)],
            )
            nc.sync.dma_start(
                out=xq8[:, NCH : NCH + NCH // 2, :].rearrange("p c k -> p (c k)"),
                in_=xht[:, ts(t, FT // 2)],
            )
            st["x_sb"] = x_sb
            st["xq8"] = xq8
            return st

        def act_cvt(st):
            """ACT converts dim-chunks 8-15 of xh -> fp8."""
            xq8, x_sb = st["xq8"], st["x_sb"]
            for h in range(2, 4):
                nc.scalar.copy(
                    xq8[:, NCH + 4 * h : NCH + 4 * h + 4, :],
                    x_sb[:, 4 * h : 4 * h + 4, :],
                )

        def S_corr_mms(st, S2_ps):
            """fp8 DoubleRow pass: (xl*2^10)@e4m3(Gh) + e4m3(xh)@(Gl*2^10)."""
            xq8 = st["xq8"]
            for i in range(NCH):
                nc.tensor.matmul(
                    S2_ps[0:MPAD, :],
                    GC_sb[:, 2 * i : 2 * i + 2, :],
                    xq8[:, 2 * i : 2 * i + 2, :],
                    start=(i == 0),
                    stop=(i == NCH - 1),
                    perf_mode=DR,
                    skip_group_check=True,
                )

        def S_merge(st, S_ps, S2_ps):
            """S_sb = main + corr * 2^-10, staged for the transposes."""
            S_c = work.tile([NSLOT + 1, TOK], F32, tag="S_c")
            nc.scalar.activation(
                S_c, S2_ps[0 : NSLOT + 1, :], func=ACTF.Copy, scale=1.0 / CSCALE
            )
            S_sb = work.tile([NSLOT + 1, TOK], F32, tag="S_sb")
            nc.vector.tensor_add(S_sb, S_ps[0 : NSLOT + 1, :], S_c)
            Stok = psT.tile([128, NQ, NSLOT + 1], F32, tag="T")
            for q in range(NQ):
                nc.tensor.transpose(
                    Stok[:, q, :],
                    S_sb[:, ts(q, 128)],
                    ident[0 : NSLOT + 1, 0 : NSLOT + 1],
                )
            st["Stok"] = Stok

        def phase_A_mm(t, st):
            """S matmuls (main fp16 + fp8 corr) -> merge -> transposes."""
            x_sb = st["x_sb"]
            S_ps = psA.tile([128, TOK], F32, tag="A")
            for c in range(NCH):
                nc.tensor.matmul(
                    S_ps[0 : NSLOT + 1, :],
                    GT_sb[:, c, :],
                    x_sb[:, c, :],
                    start=(c == 0),
                    stop=(c == NCH - 1),
                )
            S2_ps = psA2.tile([128, TOK], F32, tag="A2")
            S_corr_mms(st, S2_ps)
            S_merge(st, S_ps, S2_ps)

        def phase_B(t, st):
            """Batched softmax/gate stats; Ec' = -E*r*en, g in row 64."""
            Stok = st["Stok"]
            Etok = [
                small.tile([128, NSLOT], F32, tag=f"Etok{q}", name=f"Etok{q}")
                for q in range(NQ)
            ]
            Ec = small.tile([128, NQ, NSLOT + 1], F32, tag="Ec")
            scr = small.tile([128, NSLOT], F32, tag="scr")
            mb4 = small.tile([128, NQ], F32, tag="mb4")
            sums4 = small.tile([128, NQ], F32, tag="sums4")
            gvd4 = small.tile([128, NQ], F32, tag="gvd4")
            st4 = small.tile([128, 6, NQ], F32, tag="st4")
            mx4, r4, t4, gl4, en4, g4 = (st4[:, i, :] for i in range(6))
            cp4 = small.tile([128, NQ], F32, tag="cp4")
            gx4 = small.tile([128, NQ], F32, tag="gx4")
            nc.vector.tensor_reduce(mx4, Stok[:, :, 0:NSLOT], axis=AX_X.X, op=ALU.max)
            nc.vector.tensor_copy(gx4, Stok[:, :, NSLOT])
            nc.vector.tensor_scalar_mul(mb4, mx4, -10.0)
            for q in range(NQ):
                nc.scalar.activation(
                    Etok[q],
                    Stok[:, q, 0:NSLOT],
                    func=ACTF.Exp,
                    bias=mb4[:, q : q + 1],
                    scale=10.0,
                    accum_out=sums4[:, q : q + 1],
                )
            for q in range(NQ):
                nc.vector.tensor_mul(scr, Etok[q], gv_rep)
                nc.vector.tensor_reduce(
                    gvd4[:, q : q + 1], scr, axis=AX_X.X, op=ALU.add
                )
            nc.vector.reciprocal(r4, sums4)
            nc.vector.tensor_mul(t4, gvd4, r4)
            nc.vector.tensor_add(gl4, t4, gx4)
            nc.scalar.activation(en4, gl4, func=ACTF.Exp, bias=-gate_b, scale=-1.0)
            nc.vector.tensor_scalar_add(g4, en4, 1.0)
            nc.vector.reciprocal(g4, g4)
            nc.vector.tensor_mul(cp4, r4, en4)
            nc.vector.tensor_scalar_mul(cp4, cp4, -1.0)
            for q in range(NQ):
                nc.vector.tensor_scalar_mul(
                    Ec[:, q, 0:NSLOT], Etok[q], cp4[:, q : q + 1]
                )
            for q in range(NQ):
                nc.vector.tensor_copy(Ec[:, q, NSLOT : NSLOT + 1], g4[:, q : q + 1])
            st["Ec"] = Ec

        def phase_C_pre(t, st):
            """Ec -> slot-major (+dup to partitions 64-127), g broadcast.

            Issued BEFORE phase_A_dma(t+3) so the tiny E2-dup DMA lands on
            the sync queue ahead of the 3 MiB bulk prefetch.
            """
            Ec = st["Ec"]
            ET = psE.tile([NSLOT + 1, NQ, 128], F32, tag="E")
            for q in range(NQ):
                nc.tensor.transpose(ET[:, q, :], Ec[:, q, :], ident)
            E_sb = work.tile([NSLOT + 1, NQ, 128], F16, tag="E_sb")
            nc.scalar.copy(E_sb, ET)
            # duplicate slot rows to partitions 64-127 for the row-tiled pair
            # (idle gpsimd SWDGE queue: never queues behind bulk transfers)
            E2 = work.tile([128, NQ, 128], F16, tag="E2")
            nc.gpsimd.dma_start(out=E2[64:128, :, :], in_=E_sb[0:NSLOT, :, :])
            # broadcast the gate row (partition 64) to all partitions via a
            # ones-matmul into the psA2 bank (free between the corr-merge
            # read and the next tile's DR writes)
            g_bc = psA2.tile([128, TOK], F32, tag="A2")
            nc.tensor.matmul(
                g_bc,
                ones16[NSLOT : NSLOT + 1, :],
                E_sb.rearrange("p a b -> p (a b)")[NSLOT : NSLOT + 1, :],
                start=True,
                stop=True,
                skip_group_check=True,
            )
            g_sb = work.tile([128, TOK], F32, tag="g_sb")
            nc.scalar.copy(g_sb, g_bc)
            st["E_sb"], st["E2"], st["g_sb"] = E_sb, E2, g_sb

        def phase_C(t, st, s_next=None):
            """Row-tiled pairs: PSUM = xh + Ec'@WvN ; out = g*PSUM ; one out
            DMA per tile.

            s_next=(t2, st2): interleave tile t2's S matmuls between this
            tile's I/R matmuls so PE stays busy while DVE paces the combine.
            """
            x_sb = st["x_sb"]
            E_flat = st["E_sb"].rearrange("p a b -> p (a b)")  # [65, 512]
            E2_flat = st["E2"].rearrange("p a b -> p (a b)")  # rows 64:128
            g_sb = st["g_sb"]
            if s_next is not None:
                t2, st2 = s_next
                S_ps2 = psA.tile([128, TOK], F32, tag="A")
                st["S_ps2"] = S_ps2
                st["S2_ps2"] = psA2.tile(
                    [128, TOK], F32, tag="A2", name="S2_ps2"
                )
            o16 = opool.tile([128, NCH, TOK], F16, tag="o16")
            pend = []  # (cA, R_A, cB, R_B) with combine lagging one pair

            def drain_pair():
                cA, R_A, cB, R_B = pend.pop(0)
                nc.vector.tensor_mul(o16[:, cA, :], R_A, g_sb)
                nc.vector.tensor_mul(o16[:, cB, :], R_B, g_sb)
                if cB % 4 == 3:
                    # quarter-tile out-DMA right after its chunks complete,
                    # on the gpsimd SWDGE queue: on scalar/sync its wait for
                    # the DVE muls head-blocked everything behind it.
                    q4 = cB // 4
                    nc.gpsimd.dma_start(
                        out=outt[:, ts(t * 4 + q4, FT // 4)],
                        in_=o16[:, 4 * q4 : 4 * q4 + 4, :].rearrange(
                            "p c k -> p (c k)"
                        ),
                    )

            for j in range(NPAIR):
                cA, cB = 2 * j, 2 * j + 1
                R_A = psR.tile([128, TOK], F32, tag="R")
                R_B = psR.tile([128, TOK], F32, tag="R")
                nc.tensor.matmul(
                    R_A, ident16, x_sb[:, cA, :],
                    start=True, stop=False, skip_group_check=True,
                )
                nc.tensor.matmul(
                    R_B, ident16, x_sb[:, cB, :],
                    start=True, stop=False, skip_group_check=True,
                )
                nc.tensor.matmul(
                    R_A, WvP_sb[0:NSLOT, j, :], E_flat[0:NSLOT, :],
                    start=False, stop=True, skip_group_check=True,
                )
                nc.tensor.matmul(
                    R_B, WvP_sb[NSLOT:128, j, :], E2_flat[NSLOT:128, :],
                    start=False, stop=True, skip_group_check=True,
                )
                if s_next is not None:
                    for c in (cA, cB):
                        nc.tensor.matmul(
                            S_ps2[0 : NSLOT + 1, :],
                            GT_sb[:, c, :],
                            st2["x_sb"][:, c, :],
                            start=(c == 0),
                            stop=(c == NCH - 1),
                            skip_group_check=True,
                        )
                    xq8 = st2["xq8"]
                    S2_ps2 = st["S2_ps2"]
                    for i in (cA, cB):
                        nc.tensor.matmul(
                            S2_ps2[0:MPAD, :],
                            GC_sb[:, 2 * i : 2 * i + 2, :],
                            xq8[:, 2 * i : 2 * i + 2, :],
                            start=(i == 0),
                            stop=(i == NCH - 1),
                            perf_mode=DR,
                            skip_group_check=True,
                        )
                pend.append((cA, R_A, cB, R_B))
                if len(pend) > 1:
                    drain_pair()
            while pend:
                drain_pair()

        def phase_C_tail(t, st, s_next):
            """s_next's merge + Stok transposes (PE-queue tail)."""
            if s_next is not None:
                t2, st2 = s_next
                S_merge(st2, st["S_ps2"], st["S2_ps2"])

        # software pipeline, 4 tiles deep: x-DMA 4 tiles ahead; tile t+3's
        # S/DR matmuls interleave into tile t's combine; phase_B(t+2) —
        # whose Stok inputs were produced LAST iteration — is issued
        # before the PE-dense tail so the DVE overlaps it with the S/DR
        # block instead of colliding with the pair loop's combine muls.
        states = {}
        for u in range(4):
            states[u] = phase_A_dma(u)
        for u in range(4):
            act_cvt(states[u])
        phase_A_mm(0, states[0])
        phase_B(0, states[0])
        phase_A_mm(1, states[1])
        phase_B(1, states[1])
        phase_A_mm(2, states[2])
        phase_C_pre(0, states[0])
        for t in range(NT):
            if t + 4 < NT:
                states[t + 4] = phase_A_dma(t + 4)
            if t + 1 < NT:
                phase_C_pre(t + 1, states[t + 1])
            s_next = (t + 3, states[t + 3]) if t + 3 < NT else None
            phase_C(t, states[t], s_next=s_next)
            if t + 2 < NT:
                phase_B(t + 2, states[t + 2])
            phase_C_tail(t, states[t], s_next)
            if t + 4 < NT:
                act_cvt(states[t + 4])
            del states[t]

    nc.compile()
    return nc


def _fold_weights(memory, key_w, value_w, gate_w):
    """Fold module weights; returns un-packed (GT, GC, WvN, gv) as in v3."""
    mem = memory.astype(np.float64)
    Ws = (mem @ key_w.astype(np.float64)).astype(np.float32)  # [64, 2048]
    Wv = (mem @ value_w.astype(np.float64).T).astype(np.float32)  # [64, 2048]
    gx = np.asarray(gate_w[0, :DIM], dtype=np.float32)
    gvv = (Wv.astype(np.float64) @ gate_w[0, DIM:].astype(np.float64)).astype(
        np.float32
    )
    G = np.concatenate([Ws, gx[None, :]], axis=0)  # [65, 2048]; gate row last
    WvN = np.ascontiguousarray(-Wv).astype(np.float16)  # [64, 2048]
    F8NP = mybir.dt.np(F8)
    Gh = G.astype(np.float16)  # [65, 2048]
    Gh8 = Gh.astype(F8NP)
    Gl10 = ((G - Gh.astype(np.float32)) * CSCALE).astype(F8NP)
    GT = np.ascontiguousarray(Gh.T)  # [2048, 65] fp16
    GC = np.zeros((2 * DIM, MPAD), dtype=F8NP)
    GC[:DIM, : NSLOT + 1] = Gh8.T
    GC[DIM:, : NSLOT + 1] = Gl10.T
    return GT, GC, WvN, gvv.reshape(1, NSLOT)


def _pack_weights(GT, GC, WvN):
    """Device layouts: [128 partitions, contiguous free]."""
    # GT [2048, 65] -> [128, NCH*65]; row d = c*128+p
    GTt = np.ascontiguousarray(
        GT.reshape(NCH, 128, NSLOT + 1).transpose(1, 0, 2).reshape(128, -1)
    )
    # GC [4096, 80] -> [128, 32*80]
    GCt = np.ascontiguousarray(
        GC.reshape(2 * NCH, 128, MPAD).transpose(1, 0, 2).reshape(128, -1)
    )
    # WvN [64, 2048] -> WvP [128, NPAIR*128]
    Wv3 = WvN.reshape(NSLOT, NCH, 128)
    WvP = np.concatenate([Wv3[:, 0::2, :], Wv3[:, 1::2, :]], axis=0)
    WvP = np.ascontiguousarray(WvP.reshape(128, -1))
    return GTt, GCt, WvP


def _pack_x(xb):
    """x [L, DIM] f32 -> (xt f16, xlt f8, xht f8 [dim-chunks 0-7])."""
    F8NP = mybir.dt.np(F8)
    xT = np.ascontiguousarray(xb.T)  # [2048, 4096]
    xh = xT.astype(np.float16)
    xl8 = ((xT - xh.astype(np.float32)) * CSCALE).astype(F8NP)
    # [d= c*128+p, tok= t*512+k] -> [p, t, c, k]
    xh4 = xh.reshape(NCH, 128, NT, TOK)
    xtp = np.ascontiguousarray(xh4.transpose(1, 2, 0, 3).reshape(128, -1))
    xlp = np.ascontiguousarray(
        xl8.reshape(NCH, 128, NT, TOK).transpose(1, 2, 0, 3).reshape(128, -1)
    )
    xhp = np.ascontiguousarray(
        xh4[: NCH // 2].astype(F8NP).transpose(1, 2, 0, 3).reshape(128, -1)
    )
    return xtp, xlp, xhp


def _unpack_out(o):
    """outt [128, NT*FT] f16 -> out [L, DIM] f32."""
    # [p, t, c, k] -> [d= c*128+p, tok= t*512+k]
    oT = o.reshape(128, NT, NCH, TOK).transpose(2, 0, 1, 3).reshape(DIM, L)
    return oT.T.astype(np.float32)


def kernel(x, memory, key_w, value_w, gate_w, gate_b, _trace=False, _tmpdir=None):
    x = np.asarray(x, dtype=np.float32)
    GT, GC, WvN, gvv = _fold_weights(
        np.asarray(memory, np.float32),
        np.asarray(key_w, np.float32),
        np.asarray(value_w, np.float32),
        np.asarray(gate_w, np.float32),
    )
    GTt, GCt, WvP = _pack_weights(GT, GC, WvN)
    nc = _build(float(np.asarray(gate_b).reshape(-1)[0]))
    in_maps = []
    for b in range(B):
        xtp, xlp, xhp = _pack_x(x[b])
        in_maps.append(
            {"xt": xtp, "xlt": xlp, "xht": xhp,
             "GTt": GTt, "GCt": GCt, "WvP": WvP, "gv": gvv}
        )
    res = run_bass_kernel_spmd(
        nc, in_maps, list(range(B)), trace=_trace, tmpdir=_tmpdir
    )
    out = np.stack(
        [_unpack_out(res.results[b]["outt"]) for b in range(B)], axis=0
    )
    if _trace:
        return out, res
    return out


def sim_core0(inputs, expected):
    """CoreSim check of core 0 against expected[0]; returns maxabs err."""
    from concourse.bass_interp import CoreSim

    GT, GC, WvN, gvv = _fold_weights(
        inputs["memory"], inputs["key_w"], inputs["value_w"], inputs["gate_w"]
    )
    GTt, GCt, WvP = _pack_weights(GT, GC, WvN)
    nc = _build(float(inputs["gate_b"][0]))
    sim = CoreSim(nc)
    xtp, xlp, xhp = _pack_x(inputs["x"][0])
    sim.tensor("xt")[:] = xtp
    sim.tensor("xlt")[:] = xlp
    sim.tensor("xht")[:] = xhp
    sim.tensor("GTt")[:] = GTt
    sim.tensor("GCt")[:] = GCt
    sim.tensor("WvP")[:] = WvP
    sim.tensor("gv")[:] = gvv
    sim.simulate()
    got = _unpack_out(np.asarray(sim.tensor("outt")))
    return np.abs(got - expected[0]).max()
